# revision 52
# baseline (speedup 1.0000x reference)
"""Trainium2 Bass kernel for nn_BlocksCore (RIMs BlocksCore step).

Data-parallel over batch B=2048 across 8 NeuronCores (256 rows each),
parameters replicated. Per-core plan (v4):

  A. input attention: mask path (k1/q/s1/top-k) exact in f32; value path
     (v1T) in bf16; inp_flat^T produced feature-major as bf16 + fp8.
  B. LSTM gates all-fp8 with DoubleRow matmuls (K=256/instr), weights
     pre-scaled by 2^13 (fp8 subnormal avoidance), descaled in the PSUM
     activations; h-side fp8 residual pass on the [f|g] columns restores
     the c-path accuracy.  Processed per (hidden group g, batch half bt):
     one [128,4,256] PSUM tile per phase (3-deep rotation), weights
     fetched once per group as 16 contiguous [128,2048] lines.  Phase-C
     q/k/v projections for block g run inline right after group g's tail.
  C. communication attention: one 32-row score tile, single softmax,
     PE-expanded apply reading PSUM directly, gated residual + masked
     merge per block.

Outputs: hx_out/cx_out [256,2048] f32, mask_out [256,8] (host expands).
"""

import json
import os

import numpy as np
import ml_dtypes

BF16 = ml_dtypes.bfloat16
E4 = ml_dtypes.float8_e4m3

B = 2048
NCORES = 8
BSH = B // NCORES          # 256 batch rows per core
WSCALE = 2.0 ** 13         # fp8 weight pre-scale (keeps w out of subnormals)
WDESCALE = 2.0 ** -13
NINP = 1024
NHID = 2048
NB = 8                     # blocks
BS = 256                   # block size (NHID / NB)
DKI = 64                   # input-attention d_k

_CACHE = {}
last_exec_time_ns = None
last_results = None

# jj -> K-tile-pair order: hx pairs (8..15) first so phase B can start
# before phase A finishes producing inp_flat
JORDER = list(range(8, 16)) + list(range(8))

# ---------------------------------------------------------------------------
# BIR post-fix: this toolchain's core_v3 codegen supports only one sync-wait
# per CTRL-class instruction; hoist extras onto single-wait EventSemaphores.
# ---------------------------------------------------------------------------


def _fix_bir_json(bir_bytes: bytes) -> bytes:
    bir = json.loads(bir_bytes)
    for fn in bir.get("functions", []):
        for blk in fn.get("blocks", []):
            out = []
            for ins in blk.get("instructions", []):
                si = ins.get("sync_info") or {}
                waits = si.get("on_wait") or []
                if len(waits) > 1:
                    for j, w in enumerate(waits[:-1]):
                        out.append({
                            "name": f"{ins['name']}-w{j}",
                            "engine": ins["engine"],
                            "opcode": "EventSemaphore",
                            "ins": [],
                            "outs": [],
                            "sync_info": {"on_update": [], "on_wait": [w]},
                        })
                    si = dict(si)
                    si["on_wait"] = [waits[-1]]
                    ins = dict(ins)
                    ins["sync_info"] = si
                out.append(ins)
            blk["instructions"] = out
    return json.dumps(bir).encode()


def _install_bir_fix(nc):
    orig = nc.to_json_bytes

    def patched(*a, **k):
        return _fix_bir_json(orig(*a, **k))

    nc.to_json_bytes = patched


# ---------------------------------------------------------------------------
# Device kernel
# ---------------------------------------------------------------------------

def _build(skip_fgb):
    import concourse.bass as bass
    import concourse.tile as tile
    from concourse import mybir

    f32 = mybir.dt.float32
    bf16 = mybir.dt.bfloat16
    fp8 = mybir.dt.float8e4
    OP = mybir.AluOpType
    AF = mybir.ActivationFunctionType
    AX = mybir.AxisListType
    DR = mybir.MatmulPerfMode.DoubleRow

    nc = bass.Bass()

    # ---- I/O ------------------------------------------------------------
    inpT = nc.declare_dram_parameter("inpT", [128, 8, BSH], f32, isOutput=False)
    inpT_b = nc.declare_dram_parameter("inpT_b", [128, 8, BSH], bf16,
                                       isOutput=False)
    hxT_f = nc.declare_dram_parameter("hxT_f", [128, 16, BSH], f32,
                                      isOutput=False)
    hxT_8 = nc.declare_dram_parameter("hxT_8", [128, 16, BSH], fp8,
                                      isOutput=False)
    hxE_8 = nc.declare_dram_parameter("hxE_8", [128, 16, BSH], fp8,
                                      isOutput=False)
    hx_bm = nc.declare_dram_parameter("hx_bm", [BSH, NHID], f32, isOutput=False)
    cx_bm = nc.declare_dram_parameter("cx_bm", [BSH, NHID], f32, isOutput=False)
    wq = nc.declare_dram_parameter("wq", [128, 2, NB, DKI], f32, isOutput=False)
    wk1 = nc.declare_dram_parameter("wk1", [128, 8, DKI], f32, isOutput=False)
    wv1b = nc.declare_dram_parameter("wv1b", [128, 8, BS], bf16, isOutput=False)
    # LSTM weights, fp8*2^13: w8d[p, g, jj, plane, c] = W[k, g*1024+c] with
    # k = 128*(2*JORDER[jj]+plane)+p; per group g the 1024 columns are
    # [i|o|f|g] for hidden chunk g.
    w8d = nc.declare_dram_parameter("w8d", [128, 8, 16, 2, 1024], fp8,
                                    isOutput=False)
    bias8 = nc.declare_dram_parameter("bias8", [1, 8192], bf16, isOutput=False)
    wqc = nc.declare_dram_parameter("wqc", [128, 2, NB, 128], bf16,
                                    isOutput=False)
    wkc = nc.declare_dram_parameter("wkc", [128, 2, NB, 128], bf16,
                                    isOutput=False)
    wvc = nc.declare_dram_parameter("wvc", [128, 2, NB, 128], bf16,
                                    isOutput=False)
    fgw = nc.declare_dram_parameter("fgw", [128, 2 * BS], bf16, isOutput=False)
    fgb = nc.declare_dram_parameter("fgb", [1, 2 * BS], bf16, isOutput=False)
    hx_out = nc.declare_dram_parameter("hx_out", [BSH, NHID], f32, isOutput=True)
    cx_out = nc.declare_dram_parameter("cx_out", [BSH, NHID], f32, isOutput=True)
    mask_out = nc.declare_dram_parameter("mask_out", [BSH, NB], f32,
                                         isOutput=True)

    # ---- inline constants ----------------------------------------------
    hq_np = np.zeros((128, NB, 32), dtype=BF16)
    for d in range(128):
        for q in range(NB):
            hq_np[d, q, (d // 32) * 8 + q] = 1
    e32_np = np.zeros((32, NB, 128), dtype=BF16)
    for m in range(128):
        for q in range(NB):
            e32_np[(m // 32) * 8 + q, q, m] = 1
    # partition broadcaster: sel8[n', n, p] = (n' == n); a K=8 matmul with
    # lhsT=sel8[:, n, :] replicates row n of the rhs across 128 partitions
    sel8_np = np.zeros((8, NB, 128), dtype=BF16)
    for n in range(NB):
        sel8_np[n, n, :] = 1
    hqc = nc.inline_tensor(hq_np, "hqc")
    e32b = nc.inline_tensor(e32_np, "e32b")
    ones1c = nc.inline_tensor(np.ones((1, 128), dtype=BF16), "ones1c")
    sel8c = nc.inline_tensor(sel8_np, "sel8c")
    identc = nc.inline_tensor(np.eye(128, dtype=BF16), "identc")

    with tile.TileContext(nc) as tc:
        with tc.tile_pool(name="cp", bufs=1) as cp, \
             tc.tile_pool(name="pp", bufs=1) as pp:
            # ---- sync queue: A inputs needed earliest ------------------
            bias8_sb = cp.tile([1, 8192], bf16)
            nc.sync.dma_start(out=bias8_sb[:], in_=bias8[:])
            inpT_sb = pp.tile([128, 8, BSH], f32)
            nc.sync.dma_start(out=inpT_sb[:], in_=inpT[:])
            wk1_sb = pp.tile([128, 8, DKI], f32)
            nc.sync.dma_start(out=wk1_sb[:], in_=wk1[:])
            hxT8_sb = pp.tile([128, 16, BSH], fp8)
            nc.sync.dma_start(out=hxT8_sb[:], in_=hxT_8[:])
            hxE8_sb = pp.tile([128, 16, BSH], fp8)
            nc.sync.dma_start(out=hxE8_sb[:], in_=hxE_8[:])
            wv1_sb = pp.tile([128, 8, BS], bf16)
            nc.sync.dma_start(out=wv1_sb[:], in_=wv1b[:])
            inpTb_sb = pp.tile([128, 8, BSH], bf16)
            nc.sync.dma_start(out=inpTb_sb[:], in_=inpT_b[:])

            # ---- scalar queue: wq, then B weights join ------------------
            wq_sb = pp.tile([128, 2, NB, DKI], f32)
            nc.scalar.dma_start(out=wq_sb[:], in_=wq[:])

            # ---- gpsimd queue: hxTf first (mask path), then the rest ----
            hxTf_sb = pp.tile([128, 16, BSH], f32)
            nc.gpsimd.dma_start(out=hxTf_sb[:], in_=hxT_f[:])
            wqc_sb = cp.tile([128, 2, NB, 128], bf16)
            nc.gpsimd.dma_start(out=wqc_sb[:], in_=wqc[:])
            wkc_sb = cp.tile([128, 2, NB, 128], bf16)
            nc.gpsimd.dma_start(out=wkc_sb[:], in_=wkc[:])
            wvc_sb = cp.tile([128, 2, NB, 128], bf16)
            nc.gpsimd.dma_start(out=wvc_sb[:], in_=wvc[:])
            ident_sb = cp.tile([128, 128], bf16)
            nc.gpsimd.dma_start(out=ident_sb[:], in_=identc[:])
            sel8_sb = cp.tile([8, NB, 128], bf16)
            nc.gpsimd.dma_start(out=sel8_sb[:], in_=sel8c[:])
            ones1_sb = cp.tile([1, 128], bf16)
            nc.gpsimd.dma_start(out=ones1_sb[:], in_=ones1c[:])
            # cx/hx batch-major, loaded per-group-pair chunks in tail order
            cx_sb = [pp.tile([128, NHID], f32, tag=f"cx{bt}", name=f"cx{bt}")
                     for bt in range(2)]
            hx_sb = [pp.tile([128, NHID], f32, tag=f"hx{bt}", name=f"hx{bt}")
                     for bt in range(2)]
            for gp in range(1):
                sl = slice(gp * 512, (gp + 1) * 512)
                for bt in range(2):
                    nc.gpsimd.dma_start(out=cx_sb[bt][:, sl],
                                        in_=cx_bm[bt * 128:(bt + 1) * 128, sl])
                    nc.gpsimd.dma_start(out=hx_sb[bt][:, sl],
                                        in_=hx_bm[bt * 128:(bt + 1) * 128, sl])
            fgw_sb = cp.tile([128, 2 * BS], bf16)
            nc.gpsimd.dma_start(out=fgw_sb[:], in_=fgw[:])
            fgb_sb = cp.tile([1, 2 * BS], bf16)
            nc.gpsimd.dma_start(out=fgb_sb[:], in_=fgb[:])
            hq_sb = cp.tile([128, NB, 32], bf16)
            nc.gpsimd.dma_start(out=hq_sb[:], in_=hqc[:])
            e32_sb = cp.tile([32, NB, 128], bf16)
            nc.gpsimd.dma_start(out=e32_sb[:], in_=e32b[:])
            for gp in range(1, 4):
                sl = slice(gp * 512, (gp + 1) * 512)
                for bt in range(2):
                    nc.gpsimd.dma_start(out=cx_sb[bt][:, sl],
                                        in_=cx_bm[bt * 128:(bt + 1) * 128, sl])
                    nc.gpsimd.dma_start(out=hx_sb[bt][:, sl],
                                        in_=hx_bm[bt * 128:(bt + 1) * 128, sl])

            xt8_sb = pp.tile([128, 16, BSH], fp8)
            hnewT_sb = pp.tile([128, 16, BSH], bf16)
            mask_sb = [pp.tile([128, NB], f32, tag=f"mk{bt}", name=f"mk{bt}")
                      for bt in range(2)]
            sig_sb = [pp.tile([128, NB], bf16, tag=f"sg{bt}", name=f"sg{bt}")
                      for bt in range(2)]
            qc_sb = pp.tile([128, NB, BSH], bf16)
            kc_sb = pp.tile([128, NB, BSH], bf16)
            vc_sb = pp.tile([128, NB, BSH], bf16)

            # ---- phase A (mask path f32-exact) ---------------------------
            with tc.tile_pool(name="pa", bufs=1) as pa, \
                 tc.tile_pool(name="pa2", bufs=2) as pa2, \
                 tc.tile_pool(name="paps", bufs=1, space="PSUM") as paps:
                sigT_sb = pa.tile([8, BSH], bf16)
                for bt in range(2):
                    bsl = slice(bt * 128, (bt + 1) * 128)
                    k1_ps = paps.tile([128, DKI], f32, tag="k1")
                    for k in range(8):
                        nc.tensor.matmul(k1_ps[:], inpT_sb[:, k, bsl],
                                         wk1_sb[:, k, :],
                                         start=(k == 0), stop=(k == 7))
                    k1s = pa2.tile([128, DKI], f32, tag="k1s")
                    nc.vector.tensor_copy(k1s[:], k1_ps[:])

                    q_ps = paps.tile([128, NB, DKI], f32, tag="q")
                    for n in range(NB):
                        for s in range(2):
                            nc.tensor.matmul(q_ps[:, n, :],
                                             hxTf_sb[:, 2 * n + s, bsl],
                                             wq_sb[:, s, n, :],
                                             start=(s == 0), stop=(s == 1))
                    prod = pa2.tile([128, NB, DKI], f32, tag="prod")
                    k1a = k1s[:]
                    k1bc = bass.AP(tensor=k1a.tensor, offset=k1a.offset,
                                   ap=[k1a.ap[0], [0, NB], k1a.ap[1]])
                    nc.vector.tensor_tensor(prod[:], q_ps[:], k1bc, OP.mult)
                    s1 = pa2.tile([128, NB], f32, tag="s1")
                    nc.vector.reduce_sum(s1[:], prod[:], axis=AX.X)
                    nc.scalar.activation(sig_sb[bt][:], s1[:], AF.Sigmoid,
                                         scale=0.125)

                    # top-4 mask (rank counts fused via accum_out)
                    cnt = pa2.tile([128, NB], f32, tag="cnt")
                    tmp = pa2.tile([128, NB], f32, tag="tmp")
                    for n in range(NB):
                        nc.vector.tensor_scalar(tmp[:], s1[:], s1[:, n:n + 1],
                                                0.0, OP.is_gt, OP.add,
                                                accum_out=cnt[:, n:n + 1])
                    nc.vector.tensor_single_scalar(mask_sb[bt][:], cnt[:], 4.0,
                                                   OP.is_lt)
                    nc.gpsimd.dma_start(out=mask_out[bsl, :], in_=mask_sb[bt][:])
                    # sig^T half for the partition broadcast below
                    sgt = paps.tile([8, 128], bf16, tag="sgt")
                    nc.tensor.transpose(sgt[:], sig_sb[bt][:], ident_sb[:])
                    nc.vector.tensor_copy(sigT_sb[:, bsl], sgt[:])

                # v1^T = wv1^T @ inp^T in bf16 (value path; feeds fp8)
                v1T_sb = pa.tile([128, 2, BSH], bf16)
                for s in range(2):
                    v1T_ps = paps.tile([128, BSH], f32, tag="v1T")
                    for k in range(8):
                        nc.tensor.matmul(v1T_ps[:],
                                         wv1_sb[:, k, s * 128:(s + 1) * 128],
                                         inpTb_sb[:, k, :],
                                         start=(k == 0), stop=(k == 7))
                    nc.scalar.copy(v1T_sb[:, s, :], v1T_ps[:])

                # inp_flat^T = v1^T * broadcast(sig^T), fp8 straight out
                # of PSUM: 4 wide TTs instead of 16 mult+cast pairs
                with tc.tile_pool(name="pasg", bufs=2, space="PSUM") as pasg:
                    xa = xt8_sb[:]
                    st1 = xa.ap[1][0]
                    for nlo in (0, 4):
                        sgb = pasg.tile([128, 4, BSH], f32, tag="sgb")
                        for n in range(nlo, nlo + 4):
                            nc.tensor.matmul(sgb[:, n - nlo, :],
                                             sel8_sb[:, n, :],
                                             sigT_sb[:], start=True, stop=True)
                        for s in range(2):
                            sub = xt8_sb[:, 2 * nlo + s, :]
                            xt_v = bass.AP(tensor=sub.tensor,
                                           offset=sub.offset,
                                           ap=[sub.ap[0], [2 * st1, 4],
                                               sub.ap[-1]])
                            va = v1T_sb[:, s, :]
                            vbc = bass.AP(tensor=va.tensor, offset=va.offset,
                                          ap=[va.ap[0], [0, 4], va.ap[-1]])
                            nc.vector.tensor_tensor(xt_v, vbc, sgb[:],
                                                    OP.mult)

            # ---- phase B: LSTM groups, per (group, batch-half) ----------
            with tc.tile_pool(name="gps", bufs=1, space="PSUM") as gps, \
                 tc.tile_pool(name="prj", bufs=2, space="PSUM") as prj, \
                 tc.tile_pool(name="pw", bufs=20) as pw, \
                 tc.tile_pool(name="pb2", bufs=2) as pb2:
                w8t = {}
                for g in range(8):
                    for bt in range(2):
                        bsl = slice(bt * 128, (bt + 1) * 128)
                        gt = gps.tile([128, 4, BS], f32,
                                      tag=f"g{(2 * g + bt) % 3}",
                                      name=f"g{(2 * g + bt) % 3}")
                        nc.tensor.matmul(gt[:, 0:2, :], ones1_sb[:],
                                         bias8_sb[:, g * 1024:g * 1024 + 512],
                                         start=True, stop=False)
                        nc.tensor.matmul(gt[:, 2:4, :], ones1_sb[:],
                                         bias8_sb[:, g * 1024 + 512:
                                                  (g + 1) * 1024],
                                         start=True, stop=False)
                        for jj in range(16):
                            if bt == 0:
                                wt = pw.tile([128, 2, 1024], fp8, tag="w8t")
                                weng = nc.scalar if jj % 2 == 0 else nc.sync
                                weng.dma_start(out=wt[:], in_=w8d[:, g, jj, :, :])
                                w8t[jj] = wt
                            wt = w8t[jj]
                            st = (jj == 15)
                            if jj < 8:
                                t = 2 * jj
                                lhs8 = hxT8_sb[:, t:t + 2, bsl]
                                lhsE = hxE8_sb[:, t:t + 2, bsl]
                            else:
                                t = 2 * (jj - 8)
                                lhs8 = xt8_sb[:, t:t + 2, bsl]
                                lhsE = None
                            nc.tensor.matmul(gt[:, 0:2, :], lhs8,
                                             wt[:, :, 0:512],
                                             start=False, stop=st, perf_mode=DR)
                            nc.tensor.matmul(gt[:, 2:4, :], lhs8,
                                             wt[:, :, 512:1024],
                                             start=False, stop=st, perf_mode=DR)
                            if lhsE is not None and jj < 6:
                                nc.tensor.matmul(gt[:, 2:4, :], lhsE,
                                                 wt[:, :, 512:1024],
                                                 start=False, stop=False,
                                                 perf_mode=DR)
                        # ---- tail for (g, bt) ---------------------------
                        sl = slice(g * BS, (g + 1) * BS)
                        sio = pb2.tile([128, 2, BS], f32, tag="sio",
                                       name="sio", bufs=3)
                        nc.scalar.activation(sio[:], gt[:, 0:2, :],
                                             AF.Sigmoid, scale=WDESCALE)
                        sigf = pb2.tile([128, BS], f32, tag="sigf",
                                        name="sigf", bufs=3)
                        nc.scalar.activation(sigf[:], gt[:, 2, :],
                                             AF.Sigmoid, scale=WDESCALE)
                        tang = pb2.tile([128, BS], f32, tag="tang",
                                        name="tang", bufs=3)
                        nc.scalar.activation(tang[:], gt[:, 3, :],
                                             AF.Tanh, scale=WDESCALE)
                        t1 = pb2.tile([128, BS], f32, tag="t1", name="t1")
                        nc.vector.tensor_tensor(t1[:], sigf[:],
                                                cx_sb[bt][:, sl], OP.mult)
                        t2 = pb2.tile([128, BS], f32, tag="t2", name="t2")
                        nc.gpsimd.tensor_tensor(t2[:], sio[:, 0, :], tang[:],
                                                OP.mult)
                        cnew = pb2.tile([128, BS], f32, tag="cnew", name="cnew")
                        nc.vector.tensor_tensor(cnew[:], t1[:], t2[:], OP.add)
                        t3 = pb2.tile([128, BS], f32, tag="t3", name="t3")
                        nc.scalar.activation(t3[:], cnew[:], AF.Tanh)
                        hnb = pb2.tile([128, BS], bf16, tag="hnb", name="hnb")
                        nc.vector.tensor_tensor(hnb[:], sio[:, 1, :], t3[:],
                                                OP.mult)
                        hnw = pb2.tile([128, BS], f32, tag="hnw", name="hnw")
                        nc.gpsimd.tensor_tensor(hnw[:], sio[:, 1, :], t3[:],
                                                OP.mult)
                        dc = pb2.tile([128, BS], f32, tag="dc", name="dc")
                        nc.gpsimd.tensor_tensor(dc[:], cnew[:],
                                                cx_sb[bt][:, sl], OP.subtract)
                        co = pb2.tile([128, BS], f32, tag="co", name="co")
                        nc.vector.scalar_tensor_tensor(
                            co[:], dc[:], mask_sb[bt][:, g:g + 1],
                            cx_sb[bt][:, sl], OP.mult, OP.add)
                        nc.gpsimd.dma_start(
                            out=cx_out[bt * 128:(bt + 1) * 128, sl], in_=co[:])
                        for s in range(2):
                            teng = nc.sync if s == 0 else nc.scalar
                            teng.dma_start(
                                out=hnewT_sb[:, 2 * g + s,
                                             bt * 128:(bt + 1) * 128],
                                in_=hnb[:, s * 128:(s + 1) * 128],
                                transpose=True)
                        # d0 = h_new - hx, in place (merge shortcut)
                        nc.gpsimd.tensor_tensor(hnw[:], hnw[:],
                                                hx_sb[bt][:, sl], OP.subtract)
                        # base = mask*d0 + hx, in place in hx_sb
                        nc.vector.scalar_tensor_tensor(
                            hx_sb[bt][:, sl], hnw[:],
                            mask_sb[bt][:, g:g + 1], hx_sb[bt][:, sl],
                            OP.mult, OP.add)
                    # ---- inline phase-C projections for block g ---------
                    for wi, (wsb, osb) in enumerate(
                            ((wkc_sb, kc_sb), (wqc_sb, qc_sb),
                             (wvc_sb, vc_sb))):
                        ps = prj.tile([128, BSH], f32, tag="proj")
                        for s in range(2):
                            nc.tensor.matmul(ps[:], wsb[:, s, g, :],
                                             hnewT_sb[:, 2 * g + s, :],
                                             start=(s == 0), stop=(s == 1))
                        if wi % 2 == 0:
                            nc.scalar.copy(osb[:, g, :], ps[:])
                        else:
                            nc.vector.tensor_copy(osb[:, g, :], ps[:])

            # ============================ phase C ========================
            with tc.tile_pool(name="pc", bufs=1) as pc, \
                 tc.tile_pool(name="pctmp", bufs=2) as pctmp:
                at_sb = pc.tile([32, NB, BSH], bf16)
                with tc.tile_pool(name="psS", bufs=1, space="PSUM") as psS:
                    S = psS.tile([32, NB, BSH], f32, tag="S", name="S")
                    prg = {}
                    for q in (6, 7):
                        prg[q] = pctmp.tile([128, NB, BSH], bf16,
                                            tag=f"prg{q}", name=f"prg{q}",
                                            bufs=1)
                        qa = qc_sb[:, q, :]
                        qbc = bass.AP(tensor=qa.tensor, offset=qa.offset,
                                      ap=[qa.ap[0], [0, NB], qa.ap[-1]])
                        nc.gpsimd.tensor_tensor(prg[q][:], qbc, kc_sb[:],
                                                OP.mult)
                    for q in range(NB):
                        if q in prg:
                            pr = prg[q]
                            for kp in range(4):
                                nc.tensor.matmul(S[:, 2 * kp:2 * kp + 2, :],
                                                 hq_sb[:, q, :],
                                                 pr[:, 2 * kp:2 * kp + 2, :],
                                                 start=(q == 0), stop=(q == 7))
                        else:
                            pr = pctmp.tile([128, NB, BSH], bf16, tag="pr",
                                            name="pr", bufs=2)
                            qa = qc_sb[:, q, :]
                            qbc = bass.AP(tensor=qa.tensor, offset=qa.offset,
                                          ap=[qa.ap[0], [0, NB], qa.ap[-1]])
                            for half in range(2):
                                hs = slice(half * 4, half * 4 + 4)
                                qh = bass.AP(tensor=qa.tensor, offset=qa.offset,
                                             ap=[qa.ap[0], [0, 4], qa.ap[-1]])
                                nc.vector.tensor_tensor(pr[:, hs, :], qh,
                                                        kc_sb[:, hs, :],
                                                        OP.mult)
                                for kp in (2 * half, 2 * half + 1):
                                    nc.tensor.matmul(
                                        S[:, 2 * kp:2 * kp + 2, :],
                                        hq_sb[:, q, :],
                                        pr[:, 2 * kp:2 * kp + 2, :],
                                        start=(q == 0), stop=(q == 7))
                    ex = pc.tile([32, NB, BSH], bf16, tag="ex", name="ex")
                    nc.scalar.activation(ex[:], S[:], AF.Exp,
                                         scale=float(1.0 / np.sqrt(32.0)))
                    # denominator by bf16 tree adds (contiguous slices)
                    e1 = pctmp.tile([32, 4, BSH], bf16, tag="e1", name="e1")
                    nc.vector.tensor_tensor(e1[:], ex[:, 0:4, :], ex[:, 4:8, :],
                                            OP.add)
                    e2 = pctmp.tile([32, 2, BSH], bf16, tag="e2", name="e2")
                    nc.vector.tensor_tensor(e2[:], e1[:, 0:2, :], e1[:, 2:4, :],
                                            OP.add)
                    denom = pctmp.tile([32, BSH], f32, tag="denom",
                                       name="denom")
                    nc.vector.tensor_tensor(denom[:], e2[:, 0, :], e2[:, 1, :],
                                            OP.add)
                    recip = pctmp.tile([32, BSH], f32, tag="recip",
                                       name="recip")
                    nc.vector.reciprocal(recip[:], denom[:])
                    ra = recip[:]
                    rbc = bass.AP(tensor=ra.tensor, offset=ra.offset,
                                  ap=[ra.ap[0], [0, NB], ra.ap[-1]])
                    nc.vector.tensor_tensor(at_sb[:], ex[:], rbc, OP.mult)

                with tc.tile_pool(name="psU", bufs=1, space="PSUM") as psU, \
                     tc.tile_pool(name="psOG", bufs=2, space="PSUM") as psOG:
                    pend = []

                    def emit_merge(q, sgl, tanl):
                        qsl = slice(q * BS, (q + 1) * BS)
                        for bt in range(2):
                            mh = pctmp.tile([128, BS], f32, tag="mhq",
                                            name="mhq", bufs=4)
                            nc.vector.scalar_tensor_tensor(
                                mh[:], tanl[bt][:], mask_sb[bt][:, q:q + 1],
                                sgl[bt][:], OP.mult, OP.mult)
                            ho = pctmp.tile([128, BS], f32, tag="hoq",
                                            name="hoq", bufs=4)
                            nc.gpsimd.tensor_tensor(ho[:], mh[:],
                                                    hx_sb[bt][:, qsl], OP.add)
                            nc.gpsimd.dma_start(
                                out=hx_out[bt * 128:(bt + 1) * 128, qsl],
                                in_=ho[:])

                    for q in range(NB):
                        Ua = psU.tile([128, NB, BSH], f32, tag="Ua", name="Ua")
                        for kp in range(4):
                            nc.tensor.matmul(Ua[:, 2 * kp:2 * kp + 2, :],
                                             e32_sb[:, q, :],
                                             at_sb[:, 2 * kp:2 * kp + 2, :],
                                             start=True, stop=True)
                        m0 = pctmp.tile([128, NB, BSH], bf16, tag="m0",
                                        name="m0", bufs=2)
                        nc.vector.tensor_tensor(m0[:], Ua[:], vc_sb[:],
                                                OP.mult)
                        tr1 = pctmp.tile([128, 4, BSH], bf16, tag="tr1",
                                         name="tr1")
                        nc.vector.tensor_tensor(tr1[:], m0[:, 0:4, :],
                                                m0[:, 4:8, :], OP.add)
                        tr2 = pctmp.tile([128, 2, BSH], bf16, tag="tr2",
                                         name="tr2")
                        nc.vector.tensor_tensor(tr2[:], tr1[:, 0:2, :],
                                                tr1[:, 2:4, :], OP.add)
                        coutq = pctmp.tile([128, BSH], bf16, tag="coutq",
                                           name="coutq", bufs=2)
                        nc.vector.tensor_tensor(coutq[:], tr2[:, 0, :],
                                                tr2[:, 1, :], OP.add)
                        sgl, tanl = {}, {}
                        for bt in range(2):
                            csl = coutq[:, bt * 128:(bt + 1) * 128]
                            og = psOG.tile([128, 2 * BS], f32, tag="og",
                                           name="og")
                            nc.tensor.matmul(og[:], csl, fgw_sb[:],
                                             start=True, stop=skip_fgb)
                            if not skip_fgb:
                                nc.tensor.matmul(og[:], ones1_sb[:], fgb_sb[:],
                                                 start=False, stop=True)
                            tano = pctmp.tile([128, BS], f32, tag=f"tano{bt}",
                                              name=f"tano{bt}", bufs=2)
                            nc.scalar.activation(tano[:], og[:, 0:BS], AF.Tanh)
                            sg = pctmp.tile([128, BS], f32, tag=f"sgx{bt}",
                                            name=f"sgx{bt}", bufs=2)
                            nc.scalar.activation(sg[:], og[:, BS:2 * BS],
                                                 AF.Sigmoid)
                            sgl[bt], tanl[bt] = sg, tano
                        pend.append((q, sgl, tanl))
                        if len(pend) > 1:
                            emit_merge(*pend.pop(0))
                    while pend:
                        emit_merge(*pend.pop(0))

    _install_bir_fix(nc)
    return nc


# ---------------------------------------------------------------------------
# Host wrapper
# ---------------------------------------------------------------------------

def kernel(inp, hx, cx, wq_inp, wk_inp, wv_inp, w_ih, w_hh, b_ih, b_hh,
           wq_c, wk_c, wv_c, fc_w, fc_b, gate_w, gate_b, step=None):
    global last_exec_time_ns, last_results

    inp = np.asarray(inp, np.float32)
    hx = np.asarray(hx, np.float32)
    cx = np.asarray(cx, np.float32)
    wq_inp = np.asarray(wq_inp, np.float32)
    wk_inp = np.asarray(wk_inp, np.float32)
    wv_inp = np.asarray(wv_inp, np.float32)
    w_ih = np.asarray(w_ih, np.float32)
    w_hh = np.asarray(w_hh, np.float32)
    b_ih = np.asarray(b_ih, np.float32)
    b_hh = np.asarray(b_hh, np.float32)
    wq_c = np.asarray(wq_c, np.float32)
    wk_c = np.asarray(wk_c, np.float32)
    wv_c = np.asarray(wv_c, np.float32)
    fc_w = np.asarray(fc_w, np.float32)
    fc_b = np.asarray(fc_b, np.float32)
    gate_w = np.asarray(gate_w, np.float32)
    gate_b = np.asarray(gate_b, np.float32)

    skip_fgb = not (np.any(fc_b) or np.any(gate_b))
    key = ("nc", skip_fgb)
    if key not in _CACHE:
        _CACHE[key] = _build(skip_fgb)
    nc = _CACHE[key]

    # column permutation: per 256-wide hidden group g the fp8 panel holds
    # [i|o|f|g] columns for hidden chunk g  (torch gate order i,f,g,o)
    wcat = np.concatenate([w_ih.T, w_hh.T], axis=0)     # (4096, 8192)
    bias = (b_ih + b_hh)
    perm8 = np.concatenate([np.concatenate([
        np.arange(0 * NHID + g * BS, 0 * NHID + (g + 1) * BS),
        np.arange(3 * NHID + g * BS, 3 * NHID + (g + 1) * BS),
        np.arange(1 * NHID + g * BS, 1 * NHID + (g + 1) * BS),
        np.arange(2 * NHID + g * BS, 2 * NHID + (g + 1) * BS)])
        for g in range(8)])
    w8_np = (wcat[:, perm8] * WSCALE).astype(E4)        # (4096, 8192)
    # -> [p, g, jj, plane, c]: k = 128*(2*JORDER[jj]+plane)+p, col = g*1024+c
    tmp = w8_np.reshape(32, 128, 8, 1024)               # [ktile, p, g, c]
    kts = [2 * JORDER[jj] + pl for jj in range(16) for pl in range(2)]
    w8d = np.ascontiguousarray(
        tmp[kts].reshape(16, 2, 128, 8, 1024).transpose(2, 3, 0, 1, 4))

    shared = {
        "wq": np.ascontiguousarray(
            wq_inp.reshape(NB, 2, 128, DKI).transpose(2, 1, 0, 3)),
        "wk1": np.ascontiguousarray(
            wk_inp[1].reshape(8, 128, DKI).transpose(1, 0, 2)),
        "wv1b": np.ascontiguousarray(
            wv_inp[1].reshape(8, 128, BS).transpose(1, 0, 2).astype(BF16)),
        "w8d": w8d,
        "bias8": (bias[perm8] * WSCALE).astype(BF16).reshape(1, 8192),
        "wqc": np.ascontiguousarray(
            wq_c.astype(BF16).reshape(NB, 2, 128, 128).transpose(2, 1, 0, 3)),
        "wkc": np.ascontiguousarray(
            wk_c.astype(BF16).reshape(NB, 2, 128, 128).transpose(2, 1, 0, 3)),
        "wvc": np.ascontiguousarray(
            wv_c.astype(BF16).reshape(NB, 2, 128, 128).transpose(2, 1, 0, 3)),
        "fgw": np.ascontiguousarray(
            np.concatenate([fc_w, gate_w], axis=1)).astype(BF16),
        "fgb": np.concatenate([fc_b, gate_b]).astype(BF16).reshape(1, 2 * BS),
    }

    in_maps = []
    for c in range(NCORES):
        rs = slice(c * BSH, (c + 1) * BSH)
        inpT_c = inp[rs].T.reshape(8, 128, BSH).transpose(1, 0, 2)
        hxT = hx[rs].T.reshape(16, 128, BSH).transpose(1, 0, 2)
        hxT8 = hxT.astype(E4)
        m = {
            "inpT": np.ascontiguousarray(inpT_c),
            "inpT_b": np.ascontiguousarray(inpT_c.astype(BF16)),
            "hxT_f": np.ascontiguousarray(hxT),
            "hxT_8": np.ascontiguousarray(hxT8),
            "hxE_8": np.ascontiguousarray(
                (hxT - hxT8.astype(np.float32)).astype(E4)),
            "hx_bm": np.ascontiguousarray(hx[rs]),
            "cx_bm": np.ascontiguousarray(cx[rs]),
        }
        m.update(shared)
        in_maps.append(m)

    from concourse.bass_utils import run_bass_kernel_spmd
    trace = bool(int(os.environ.get("BASS_KTRACE", "0")))
    res = run_bass_kernel_spmd(nc, in_maps, list(range(NCORES)), trace=trace)
    last_exec_time_ns = res.exec_time_ns
    last_results = res

    hx_full = np.empty((B, NHID), np.float32)
    cx_full = np.empty((B, NHID), np.float32)
    mask_full = np.empty((B, NHID), np.float32)
    for c in range(NCORES):
        rs = slice(c * BSH, (c + 1) * BSH)
        hx_full[rs] = res.results[c]["hx_out"]
        cx_full[rs] = res.results[c]["cx_out"]
        mask_full[rs] = np.repeat(res.results[c]["mask_out"], BS, axis=1)
    return hx_full, cx_full, mask_full


# revision 53
# speedup vs baseline: 1.1646x; 1.1646x over previous
"""Trainium2 Bass kernel for nn_BlocksCore (RIMs BlocksCore step).

Data-parallel over batch B=2048 across 8 NeuronCores (256 rows each),
parameters replicated. Per-core plan (v4):

  A. input attention: mask path (k1/q/s1/top-k) exact in f32; value path
     (v1T) in bf16; inp_flat^T produced feature-major as bf16 + fp8.
  B. LSTM gates all-fp8 with DoubleRow matmuls (K=256/instr), weights
     pre-scaled by 2^13 (fp8 subnormal avoidance), descaled in the PSUM
     activations; h-side fp8 residual pass on the [f|g] columns restores
     the c-path accuracy.  Processed per (hidden group g, batch half bt):
     one [128,4,256] PSUM tile per phase (3-deep rotation), weights
     fetched once per group as 16 contiguous [128,2048] lines.  Phase-C
     q/k/v projections for block g run inline right after group g's tail.
  C. communication attention: one 32-row score tile, single softmax,
     PE-expanded apply reading PSUM directly, gated residual + masked
     merge per block.

Outputs: hx_out/cx_out [256,2048] f32, mask_out [256,8] (host expands).
"""

import json
import os

import numpy as np
import ml_dtypes

BF16 = ml_dtypes.bfloat16
E4 = ml_dtypes.float8_e4m3

B = 2048
NCORES = 8
BSH = B // NCORES          # 256 batch rows per core
WSCALE = 2.0 ** 13         # fp8 weight pre-scale (keeps w out of subnormals)
WDESCALE = 2.0 ** -13
NINP = 1024
NHID = 2048
NB = 8                     # blocks
BS = 256                   # block size (NHID / NB)
DKI = 64                   # input-attention d_k

_CACHE = {}
last_exec_time_ns = None
last_results = None

# jj -> K-tile-pair order: hx pairs (8..15) first so phase B can start
# before phase A finishes producing inp_flat
JORDER = list(range(8, 16)) + list(range(8))

# ---------------------------------------------------------------------------
# BIR post-fix: this toolchain's core_v3 codegen supports only one sync-wait
# per CTRL-class instruction; hoist extras onto single-wait EventSemaphores.
# ---------------------------------------------------------------------------


def _fix_bir_json(bir_bytes: bytes) -> bytes:
    bir = json.loads(bir_bytes)
    for fn in bir.get("functions", []):
        for blk in fn.get("blocks", []):
            out = []
            for ins in blk.get("instructions", []):
                si = ins.get("sync_info") or {}
                waits = si.get("on_wait") or []
                if len(waits) > 1:
                    for j, w in enumerate(waits[:-1]):
                        out.append({
                            "name": f"{ins['name']}-w{j}",
                            "engine": ins["engine"],
                            "opcode": "EventSemaphore",
                            "ins": [],
                            "outs": [],
                            "sync_info": {"on_update": [], "on_wait": [w]},
                        })
                    si = dict(si)
                    si["on_wait"] = [waits[-1]]
                    ins = dict(ins)
                    ins["sync_info"] = si
                out.append(ins)
            blk["instructions"] = out
    return json.dumps(bir).encode()


def _install_bir_fix(nc):
    orig = nc.to_json_bytes

    def patched(*a, **k):
        return _fix_bir_json(orig(*a, **k))

    nc.to_json_bytes = patched


# ---------------------------------------------------------------------------
# Device kernel
# ---------------------------------------------------------------------------

def _build(skip_fgb):
    import concourse.bass as bass
    import concourse.tile as tile
    from concourse import mybir

    f32 = mybir.dt.float32
    bf16 = mybir.dt.bfloat16
    fp8 = mybir.dt.float8e4
    OP = mybir.AluOpType
    AF = mybir.ActivationFunctionType
    AX = mybir.AxisListType
    DR = mybir.MatmulPerfMode.DoubleRow

    nc = bass.Bass()

    # ---- I/O ------------------------------------------------------------
    inpT = nc.declare_dram_parameter("inpT", [128, 8, BSH], f32, isOutput=False)
    inpT_b = nc.declare_dram_parameter("inpT_b", [128, 8, BSH], bf16,
                                       isOutput=False)
    hxT_f = nc.declare_dram_parameter("hxT_f", [128, 16, BSH], f32,
                                      isOutput=False)
    hxT_8 = nc.declare_dram_parameter("hxT_8", [128, 16, BSH], fp8,
                                      isOutput=False)
    hxE_8 = nc.declare_dram_parameter("hxE_8", [128, 16, BSH], fp8,
                                      isOutput=False)
    hx_bm = nc.declare_dram_parameter("hx_bm", [BSH, NHID], f32, isOutput=False)
    cx_bm = nc.declare_dram_parameter("cx_bm", [BSH, NHID], f32, isOutput=False)
    wq = nc.declare_dram_parameter("wq", [128, 2, NB, DKI], f32, isOutput=False)
    wk1 = nc.declare_dram_parameter("wk1", [128, 8, DKI], f32, isOutput=False)
    wv1b = nc.declare_dram_parameter("wv1b", [128, 8, BS], bf16, isOutput=False)
    # LSTM weights, fp8*2^13: w8d[p, g, jj, plane, c] = W[k, g*1024+c] with
    # k = 128*(2*JORDER[jj]+plane)+p; per group g the 1024 columns are
    # [i|o|f|g] for hidden chunk g.
    w8d = nc.declare_dram_parameter("w8d", [128, 8, 16, 2, 1024], fp8,
                                    isOutput=False)
    bias8 = nc.declare_dram_parameter("bias8", [1, 8192], bf16, isOutput=False)
    wqc = nc.declare_dram_parameter("wqc", [128, 2, NB, 128], bf16,
                                    isOutput=False)
    wkc = nc.declare_dram_parameter("wkc", [128, 2, NB, 128], bf16,
                                    isOutput=False)
    wvc = nc.declare_dram_parameter("wvc", [128, 2, NB, 128], bf16,
                                    isOutput=False)
    fgw = nc.declare_dram_parameter("fgw", [128, 2 * BS], bf16, isOutput=False)
    fgb = nc.declare_dram_parameter("fgb", [1, 2 * BS], bf16, isOutput=False)
    hx_out = nc.declare_dram_parameter("hx_out", [BSH, NHID], f32, isOutput=True)
    cx_out = nc.declare_dram_parameter("cx_out", [BSH, NHID], f32, isOutput=True)
    mask_out = nc.declare_dram_parameter("mask_out", [BSH, NB], f32,
                                         isOutput=True)

    # ---- inline constants ----------------------------------------------
    hq_np = np.zeros((128, NB, 32), dtype=BF16)
    for d in range(128):
        for q in range(NB):
            hq_np[d, q, (d // 32) * 8 + q] = 1
    e32_np = np.zeros((32, NB, 128), dtype=BF16)
    for m in range(128):
        for q in range(NB):
            e32_np[(m // 32) * 8 + q, q, m] = 1
    # partition broadcaster: sel8[n', n, p] = (n' == n); a K=8 matmul with
    # lhsT=sel8[:, n, :] replicates row n of the rhs across 128 partitions
    sel8_np = np.zeros((8, NB, 128), dtype=BF16)
    for n in range(NB):
        sel8_np[n, n, :] = 1
    hqc = nc.inline_tensor(hq_np, "hqc")
    e32b = nc.inline_tensor(e32_np, "e32b")
    ones1c = nc.inline_tensor(np.ones((1, 128), dtype=BF16), "ones1c")
    sel8c = nc.inline_tensor(sel8_np, "sel8c")
    identc = nc.inline_tensor(np.eye(128, dtype=BF16), "identc")

    with tile.TileContext(nc) as tc:
        with tc.tile_pool(name="cp", bufs=1) as cp, \
             tc.tile_pool(name="pp", bufs=1) as pp:
            # ---- sync queue: A inputs needed earliest ------------------
            bias8_sb = cp.tile([1, 8192], bf16)
            nc.sync.dma_start(out=bias8_sb[:], in_=bias8[:])
            inpT_sb = pp.tile([128, 8, BSH], f32)
            nc.sync.dma_start(out=inpT_sb[:], in_=inpT[:])
            wk1_sb = pp.tile([128, 8, DKI], f32)
            nc.sync.dma_start(out=wk1_sb[:], in_=wk1[:])
            hxT8_sb = pp.tile([128, 16, BSH], fp8)
            nc.sync.dma_start(out=hxT8_sb[:], in_=hxT_8[:])
            hxE8_sb = pp.tile([128, 16, BSH], fp8)
            nc.sync.dma_start(out=hxE8_sb[:], in_=hxE_8[:])
            wv1_sb = pp.tile([128, 8, BS], bf16)
            nc.sync.dma_start(out=wv1_sb[:], in_=wv1b[:])
            inpTb_sb = pp.tile([128, 8, BSH], bf16)
            nc.sync.dma_start(out=inpTb_sb[:], in_=inpT_b[:])

            # ---- scalar queue: wq, then B weights join ------------------
            wq_sb = pp.tile([128, 2, NB, DKI], f32)
            nc.scalar.dma_start(out=wq_sb[:], in_=wq[:])

            # ---- gpsimd queue: hxTf first (mask path), then the rest ----
            hxTf_sb = pp.tile([128, 16, BSH], f32)
            nc.gpsimd.dma_start(out=hxTf_sb[:], in_=hxT_f[:])
            wqc_sb = cp.tile([128, 2, NB, 128], bf16)
            nc.gpsimd.dma_start(out=wqc_sb[:], in_=wqc[:])
            wkc_sb = cp.tile([128, 2, NB, 128], bf16)
            nc.gpsimd.dma_start(out=wkc_sb[:], in_=wkc[:])
            wvc_sb = cp.tile([128, 2, NB, 128], bf16)
            nc.gpsimd.dma_start(out=wvc_sb[:], in_=wvc[:])
            ident_sb = cp.tile([128, 128], bf16)
            nc.gpsimd.dma_start(out=ident_sb[:], in_=identc[:])
            sel8_sb = cp.tile([8, NB, 128], bf16)
            nc.gpsimd.dma_start(out=sel8_sb[:], in_=sel8c[:])
            ones1_sb = cp.tile([1, 128], bf16)
            nc.gpsimd.dma_start(out=ones1_sb[:], in_=ones1c[:])
            # cx/hx batch-major, loaded per-group-pair chunks in tail order
            cx_sb = [pp.tile([128, NHID], f32, tag=f"cx{bt}", name=f"cx{bt}")
                     for bt in range(2)]
            hx_sb = [pp.tile([128, NHID], f32, tag=f"hx{bt}", name=f"hx{bt}")
                     for bt in range(2)]
            for gp in range(1):
                sl = slice(gp * 512, (gp + 1) * 512)
                for bt in range(2):
                    nc.gpsimd.dma_start(out=cx_sb[bt][:, sl],
                                        in_=cx_bm[bt * 128:(bt + 1) * 128, sl])
                    nc.gpsimd.dma_start(out=hx_sb[bt][:, sl],
                                        in_=hx_bm[bt * 128:(bt + 1) * 128, sl])
            fgw_sb = cp.tile([128, 2 * BS], bf16)
            nc.gpsimd.dma_start(out=fgw_sb[:], in_=fgw[:])
            fgb_sb = cp.tile([1, 2 * BS], bf16)
            nc.gpsimd.dma_start(out=fgb_sb[:], in_=fgb[:])
            hq_sb = cp.tile([128, NB, 32], bf16)
            nc.gpsimd.dma_start(out=hq_sb[:], in_=hqc[:])
            e32_sb = cp.tile([32, NB, 128], bf16)
            nc.gpsimd.dma_start(out=e32_sb[:], in_=e32b[:])
            for gp in range(1, 4):
                sl = slice(gp * 512, (gp + 1) * 512)
                for bt in range(2):
                    nc.gpsimd.dma_start(out=cx_sb[bt][:, sl],
                                        in_=cx_bm[bt * 128:(bt + 1) * 128, sl])
                    nc.gpsimd.dma_start(out=hx_sb[bt][:, sl],
                                        in_=hx_bm[bt * 128:(bt + 1) * 128, sl])

            xt8_sb = pp.tile([128, 16, BSH], fp8)
            hnewT_sb = pp.tile([128, 16, BSH], bf16)
            mask_sb = [pp.tile([128, NB], f32, tag=f"mk{bt}", name=f"mk{bt}")
                      for bt in range(2)]
            sig_sb = [pp.tile([128, NB], bf16, tag=f"sg{bt}", name=f"sg{bt}")
                      for bt in range(2)]
            qc_sb = pp.tile([128, NB, BSH], bf16)
            kc_sb = pp.tile([128, NB, BSH], bf16)
            vc_sb = pp.tile([128, NB, BSH], bf16)

            # ---- phase A (mask path f32-exact) ---------------------------
            with tc.tile_pool(name="pa", bufs=1) as pa, \
                 tc.tile_pool(name="pa2", bufs=2) as pa2, \
                 tc.tile_pool(name="paps", bufs=1, space="PSUM") as paps:
                sigT_sb = pa.tile([8, BSH], bf16)
                for bt in range(2):
                    bsl = slice(bt * 128, (bt + 1) * 128)
                    k1_ps = paps.tile([128, DKI], f32, tag="k1")
                    for k in range(8):
                        nc.tensor.matmul(k1_ps[:], inpT_sb[:, k, bsl],
                                         wk1_sb[:, k, :],
                                         start=(k == 0), stop=(k == 7))
                    k1s = pa2.tile([128, DKI], f32, tag="k1s")
                    nc.vector.tensor_copy(k1s[:], k1_ps[:])

                    q_ps = paps.tile([128, NB, DKI], f32, tag="q")
                    for n in range(NB):
                        for s in range(2):
                            nc.tensor.matmul(q_ps[:, n, :],
                                             hxTf_sb[:, 2 * n + s, bsl],
                                             wq_sb[:, s, n, :],
                                             start=(s == 0), stop=(s == 1))
                    prod = pa2.tile([128, NB, DKI], f32, tag="prod")
                    k1a = k1s[:]
                    k1bc = bass.AP(tensor=k1a.tensor, offset=k1a.offset,
                                   ap=[k1a.ap[0], [0, NB], k1a.ap[1]])
                    nc.vector.tensor_tensor(prod[:], q_ps[:], k1bc, OP.mult)
                    s1 = pa2.tile([128, NB], f32, tag="s1")
                    nc.vector.reduce_sum(s1[:], prod[:], axis=AX.X)
                    nc.scalar.activation(sig_sb[bt][:], s1[:], AF.Sigmoid,
                                         scale=0.125)

                    # top-4 mask (rank counts fused via accum_out)
                    cnt = pa2.tile([128, NB], f32, tag="cnt")
                    tmp = pa2.tile([128, NB], f32, tag="tmp")
                    for n in range(NB):
                        nc.vector.tensor_scalar(tmp[:], s1[:], s1[:, n:n + 1],
                                                0.0, OP.is_gt, OP.add,
                                                accum_out=cnt[:, n:n + 1])
                    nc.vector.tensor_single_scalar(mask_sb[bt][:], cnt[:], 4.0,
                                                   OP.is_lt)
                    nc.gpsimd.dma_start(out=mask_out[bsl, :], in_=mask_sb[bt][:])
                    # sig^T half for the partition broadcast below
                    sgt = paps.tile([8, 128], bf16, tag="sgt")
                    nc.tensor.transpose(sgt[:], sig_sb[bt][:], ident_sb[:])
                    nc.vector.tensor_copy(sigT_sb[:, bsl], sgt[:])

                # v1^T = wv1^T @ inp^T in bf16 (value path; feeds fp8)
                v1T_sb = pa.tile([128, 2, BSH], bf16)
                for s in range(2):
                    v1T_ps = paps.tile([128, BSH], f32, tag="v1T")
                    for k in range(8):
                        nc.tensor.matmul(v1T_ps[:],
                                         wv1_sb[:, k, s * 128:(s + 1) * 128],
                                         inpTb_sb[:, k, :],
                                         start=(k == 0), stop=(k == 7))
                    nc.scalar.copy(v1T_sb[:, s, :], v1T_ps[:])

                # inp_flat^T = v1^T * broadcast(sig^T), fp8 straight out
                # of PSUM: 4 wide TTs instead of 16 mult+cast pairs
                with tc.tile_pool(name="pasg", bufs=2, space="PSUM") as pasg:
                    xa = xt8_sb[:]
                    st1 = xa.ap[1][0]
                    for nlo in (0, 4):
                        sgb = pasg.tile([128, 4, BSH], f32, tag="sgb")
                        for n in range(nlo, nlo + 4):
                            nc.tensor.matmul(sgb[:, n - nlo, :],
                                             sel8_sb[:, n, :],
                                             sigT_sb[:], start=True, stop=True)
                        for s in range(2):
                            sub = xt8_sb[:, 2 * nlo + s, :]
                            xt_v = bass.AP(tensor=sub.tensor,
                                           offset=sub.offset,
                                           ap=[sub.ap[0], [2 * st1, 4],
                                               sub.ap[-1]])
                            va = v1T_sb[:, s, :]
                            vbc = bass.AP(tensor=va.tensor, offset=va.offset,
                                          ap=[va.ap[0], [0, 4], va.ap[-1]])
                            nc.vector.tensor_tensor(xt_v, vbc, sgb[:],
                                                    OP.mult)

            # ---- phase B: LSTM groups, per (group, batch-half) ----------
            with tc.tile_pool(name="gps", bufs=1, space="PSUM") as gps, \
                 tc.tile_pool(name="prj", bufs=2, space="PSUM") as prj, \
                 tc.tile_pool(name="pw", bufs=20) as pw, \
                 tc.tile_pool(name="pb2", bufs=2) as pb2:
                w8t = {}
                for g in range(8):
                    for bt in range(2):
                        bsl = slice(bt * 128, (bt + 1) * 128)
                        gt = gps.tile([128, 4, BS], f32,
                                      tag=f"g{(2 * g + bt) % 3}",
                                      name=f"g{(2 * g + bt) % 3}")
                        nc.tensor.matmul(gt[:, 0:2, :], ones1_sb[:],
                                         bias8_sb[:, g * 1024:g * 1024 + 512],
                                         start=True, stop=False)
                        nc.tensor.matmul(gt[:, 2:4, :], ones1_sb[:],
                                         bias8_sb[:, g * 1024 + 512:
                                                  (g + 1) * 1024],
                                         start=True, stop=False)
                        for jj in range(16):
                            if bt == 0:
                                wt = pw.tile([128, 2, 1024], fp8, tag="w8t")
                                weng = nc.scalar if jj % 2 == 0 else nc.sync
                                weng.dma_start(out=wt[:], in_=w8d[:, g, jj, :, :])
                                w8t[jj] = wt
                            wt = w8t[jj]
                            st = (jj == 15)
                            if jj < 8:
                                t = 2 * jj
                                lhs8 = hxT8_sb[:, t:t + 2, bsl]
                                lhsE = hxE8_sb[:, t:t + 2, bsl]
                            else:
                                t = 2 * (jj - 8)
                                lhs8 = xt8_sb[:, t:t + 2, bsl]
                                lhsE = None
                            nc.tensor.matmul(gt[:, 0:2, :], lhs8,
                                             wt[:, :, 0:512],
                                             start=False, stop=st, perf_mode=DR)
                            nc.tensor.matmul(gt[:, 2:4, :], lhs8,
                                             wt[:, :, 512:1024],
                                             start=False, stop=st, perf_mode=DR)
                            if lhsE is not None:
                                nc.tensor.matmul(gt[:, 2:4, :], lhsE,
                                                 wt[:, :, 512:1024],
                                                 start=False, stop=False,
                                                 perf_mode=DR)
                        # ---- tail for (g, bt) ---------------------------
                        sl = slice(g * BS, (g + 1) * BS)
                        sio = pb2.tile([128, 2, BS], f32, tag="sio",
                                       name="sio", bufs=3)
                        nc.scalar.activation(sio[:], gt[:, 0:2, :],
                                             AF.Sigmoid, scale=WDESCALE)
                        sigf = pb2.tile([128, BS], f32, tag="sigf",
                                        name="sigf", bufs=3)
                        nc.scalar.activation(sigf[:], gt[:, 2, :],
                                             AF.Sigmoid, scale=WDESCALE)
                        tang = pb2.tile([128, BS], f32, tag="tang",
                                        name="tang", bufs=3)
                        nc.scalar.activation(tang[:], gt[:, 3, :],
                                             AF.Tanh, scale=WDESCALE)
                        t1 = pb2.tile([128, BS], f32, tag="t1", name="t1")
                        nc.vector.tensor_tensor(t1[:], sigf[:],
                                                cx_sb[bt][:, sl], OP.mult)
                        t2 = pb2.tile([128, BS], f32, tag="t2", name="t2")
                        nc.gpsimd.tensor_tensor(t2[:], sio[:, 0, :], tang[:],
                                                OP.mult)
                        cnew = pb2.tile([128, BS], f32, tag="cnew", name="cnew")
                        nc.vector.tensor_tensor(cnew[:], t1[:], t2[:], OP.add)
                        t3 = pb2.tile([128, BS], f32, tag="t3", name="t3")
                        nc.scalar.activation(t3[:], cnew[:], AF.Tanh)
                        hnb = pb2.tile([128, BS], bf16, tag="hnb", name="hnb")
                        nc.vector.tensor_tensor(hnb[:], sio[:, 1, :], t3[:],
                                                OP.mult)
                        hnw = pb2.tile([128, BS], f32, tag="hnw", name="hnw")
                        nc.gpsimd.tensor_tensor(hnw[:], sio[:, 1, :], t3[:],
                                                OP.mult)
                        dc = pb2.tile([128, BS], f32, tag="dc", name="dc")
                        nc.gpsimd.tensor_tensor(dc[:], cnew[:],
                                                cx_sb[bt][:, sl], OP.subtract)
                        co = pb2.tile([128, BS], f32, tag="co", name="co")
                        nc.vector.scalar_tensor_tensor(
                            co[:], dc[:], mask_sb[bt][:, g:g + 1],
                            cx_sb[bt][:, sl], OP.mult, OP.add)
                        nc.gpsimd.dma_start(
                            out=cx_out[bt * 128:(bt + 1) * 128, sl], in_=co[:])
                        for s in range(2):
                            teng = nc.sync if s == 0 else nc.scalar
                            teng.dma_start(
                                out=hnewT_sb[:, 2 * g + s,
                                             bt * 128:(bt + 1) * 128],
                                in_=hnb[:, s * 128:(s + 1) * 128],
                                transpose=True)
                        # d0 = h_new - hx, in place (merge shortcut)
                        nc.gpsimd.tensor_tensor(hnw[:], hnw[:],
                                                hx_sb[bt][:, sl], OP.subtract)
                        # base = mask*d0 + hx, in place in hx_sb
                        nc.vector.scalar_tensor_tensor(
                            hx_sb[bt][:, sl], hnw[:],
                            mask_sb[bt][:, g:g + 1], hx_sb[bt][:, sl],
                            OP.mult, OP.add)
                    # ---- inline phase-C projections for block g ---------
                    for wi, (wsb, osb) in enumerate(
                            ((wkc_sb, kc_sb), (wqc_sb, qc_sb),
                             (wvc_sb, vc_sb))):
                        ps = prj.tile([128, BSH], f32, tag="proj")
                        for s in range(2):
                            nc.tensor.matmul(ps[:], wsb[:, s, g, :],
                                             hnewT_sb[:, 2 * g + s, :],
                                             start=(s == 0), stop=(s == 1))
                        if wi % 2 == 0:
                            nc.scalar.copy(osb[:, g, :], ps[:])
                        else:
                            nc.vector.tensor_copy(osb[:, g, :], ps[:])

            # ============================ phase C ========================
            with tc.tile_pool(name="pc", bufs=1) as pc, \
                 tc.tile_pool(name="pctmp", bufs=2) as pctmp:
                at_sb = pc.tile([32, NB, BSH], bf16)
                with tc.tile_pool(name="psS", bufs=1, space="PSUM") as psS:
                    S = psS.tile([32, NB, BSH], f32, tag="S", name="S")
                    prg = {}
                    for q in (6, 7):
                        prg[q] = pctmp.tile([128, NB, BSH], bf16,
                                            tag=f"prg{q}", name=f"prg{q}",
                                            bufs=1)
                        qa = qc_sb[:, q, :]
                        qbc = bass.AP(tensor=qa.tensor, offset=qa.offset,
                                      ap=[qa.ap[0], [0, NB], qa.ap[-1]])
                        nc.gpsimd.tensor_tensor(prg[q][:], qbc, kc_sb[:],
                                                OP.mult)
                    for q in range(NB):
                        if q in prg:
                            pr = prg[q]
                            for kp in range(4):
                                nc.tensor.matmul(S[:, 2 * kp:2 * kp + 2, :],
                                                 hq_sb[:, q, :],
                                                 pr[:, 2 * kp:2 * kp + 2, :],
                                                 start=(q == 0), stop=(q == 7))
                        else:
                            pr = pctmp.tile([128, NB, BSH], bf16, tag="pr",
                                            name="pr", bufs=2)
                            qa = qc_sb[:, q, :]
                            qbc = bass.AP(tensor=qa.tensor, offset=qa.offset,
                                          ap=[qa.ap[0], [0, NB], qa.ap[-1]])
                            for half in range(2):
                                hs = slice(half * 4, half * 4 + 4)
                                qh = bass.AP(tensor=qa.tensor, offset=qa.offset,
                                             ap=[qa.ap[0], [0, 4], qa.ap[-1]])
                                nc.vector.tensor_tensor(pr[:, hs, :], qh,
                                                        kc_sb[:, hs, :],
                                                        OP.mult)
                                for kp in (2 * half, 2 * half + 1):
                                    nc.tensor.matmul(
                                        S[:, 2 * kp:2 * kp + 2, :],
                                        hq_sb[:, q, :],
                                        pr[:, 2 * kp:2 * kp + 2, :],
                                        start=(q == 0), stop=(q == 7))
                    ex = pc.tile([32, NB, BSH], bf16, tag="ex", name="ex")
                    nc.scalar.activation(ex[:], S[:], AF.Exp,
                                         scale=float(1.0 / np.sqrt(32.0)))
                    # denominator by bf16 tree adds (contiguous slices)
                    e1 = pctmp.tile([32, 4, BSH], bf16, tag="e1", name="e1")
                    nc.vector.tensor_tensor(e1[:], ex[:, 0:4, :], ex[:, 4:8, :],
                                            OP.add)
                    e2 = pctmp.tile([32, 2, BSH], bf16, tag="e2", name="e2")
                    nc.vector.tensor_tensor(e2[:], e1[:, 0:2, :], e1[:, 2:4, :],
                                            OP.add)
                    denom = pctmp.tile([32, BSH], f32, tag="denom",
                                       name="denom")
                    nc.vector.tensor_tensor(denom[:], e2[:, 0, :], e2[:, 1, :],
                                            OP.add)
                    recip = pctmp.tile([32, BSH], f32, tag="recip",
                                       name="recip")
                    nc.vector.reciprocal(recip[:], denom[:])
                    ra = recip[:]
                    rbc = bass.AP(tensor=ra.tensor, offset=ra.offset,
                                  ap=[ra.ap[0], [0, NB], ra.ap[-1]])
                    nc.vector.tensor_tensor(at_sb[:], ex[:], rbc, OP.mult)

                with tc.tile_pool(name="psU", bufs=1, space="PSUM") as psU, \
                     tc.tile_pool(name="psOG", bufs=2, space="PSUM") as psOG:
                    pend = []

                    def emit_merge(q, sgl, tanl):
                        qsl = slice(q * BS, (q + 1) * BS)
                        for bt in range(2):
                            mh = pctmp.tile([128, BS], f32, tag="mhq",
                                            name="mhq", bufs=4)
                            nc.vector.scalar_tensor_tensor(
                                mh[:], tanl[bt][:], mask_sb[bt][:, q:q + 1],
                                sgl[bt][:], OP.mult, OP.mult)
                            ho = pctmp.tile([128, BS], f32, tag="hoq",
                                            name="hoq", bufs=4)
                            nc.gpsimd.tensor_tensor(ho[:], mh[:],
                                                    hx_sb[bt][:, qsl], OP.add)
                            nc.gpsimd.dma_start(
                                out=hx_out[bt * 128:(bt + 1) * 128, qsl],
                                in_=ho[:])

                    for q in range(NB):
                        Ua = psU.tile([128, NB, BSH], f32, tag="Ua", name="Ua")
                        for kp in range(4):
                            nc.tensor.matmul(Ua[:, 2 * kp:2 * kp + 2, :],
                                             e32_sb[:, q, :],
                                             at_sb[:, 2 * kp:2 * kp + 2, :],
                                             start=True, stop=True)
                        m0 = pctmp.tile([128, NB, BSH], bf16, tag="m0",
                                        name="m0", bufs=2)
                        nc.vector.tensor_tensor(m0[:], Ua[:], vc_sb[:],
                                                OP.mult)
                        tr1 = pctmp.tile([128, 4, BSH], bf16, tag="tr1",
                                         name="tr1")
                        nc.vector.tensor_tensor(tr1[:], m0[:, 0:4, :],
                                                m0[:, 4:8, :], OP.add)
                        tr2 = pctmp.tile([128, 2, BSH], bf16, tag="tr2",
                                         name="tr2")
                        nc.vector.tensor_tensor(tr2[:], tr1[:, 0:2, :],
                                                tr1[:, 2:4, :], OP.add)
                        coutq = pctmp.tile([128, BSH], bf16, tag="coutq",
                                           name="coutq", bufs=2)
                        nc.vector.tensor_tensor(coutq[:], tr2[:, 0, :],
                                                tr2[:, 1, :], OP.add)
                        sgl, tanl = {}, {}
                        for bt in range(2):
                            csl = coutq[:, bt * 128:(bt + 1) * 128]
                            og = psOG.tile([128, 2 * BS], f32, tag="og",
                                           name="og")
                            nc.tensor.matmul(og[:], csl, fgw_sb[:],
                                             start=True, stop=skip_fgb)
                            if not skip_fgb:
                                nc.tensor.matmul(og[:], ones1_sb[:], fgb_sb[:],
                                                 start=False, stop=True)
                            tano = pctmp.tile([128, BS], f32, tag=f"tano{bt}",
                                              name=f"tano{bt}", bufs=2)
                            nc.scalar.activation(tano[:], og[:, 0:BS], AF.Tanh)
                            sg = pctmp.tile([128, BS], f32, tag=f"sgx{bt}",
                                            name=f"sgx{bt}", bufs=2)
                            nc.scalar.activation(sg[:], og[:, BS:2 * BS],
                                                 AF.Sigmoid)
                            sgl[bt], tanl[bt] = sg, tano
                        pend.append((q, sgl, tanl))
                        if len(pend) > 1:
                            emit_merge(*pend.pop(0))
                    while pend:
                        emit_merge(*pend.pop(0))

    _install_bir_fix(nc)
    return nc


# ---------------------------------------------------------------------------
# Host wrapper
# ---------------------------------------------------------------------------

def kernel(inp, hx, cx, wq_inp, wk_inp, wv_inp, w_ih, w_hh, b_ih, b_hh,
           wq_c, wk_c, wv_c, fc_w, fc_b, gate_w, gate_b, step=None):
    global last_exec_time_ns, last_results

    inp = np.asarray(inp, np.float32)
    hx = np.asarray(hx, np.float32)
    cx = np.asarray(cx, np.float32)
    wq_inp = np.asarray(wq_inp, np.float32)
    wk_inp = np.asarray(wk_inp, np.float32)
    wv_inp = np.asarray(wv_inp, np.float32)
    w_ih = np.asarray(w_ih, np.float32)
    w_hh = np.asarray(w_hh, np.float32)
    b_ih = np.asarray(b_ih, np.float32)
    b_hh = np.asarray(b_hh, np.float32)
    wq_c = np.asarray(wq_c, np.float32)
    wk_c = np.asarray(wk_c, np.float32)
    wv_c = np.asarray(wv_c, np.float32)
    fc_w = np.asarray(fc_w, np.float32)
    fc_b = np.asarray(fc_b, np.float32)
    gate_w = np.asarray(gate_w, np.float32)
    gate_b = np.asarray(gate_b, np.float32)

    skip_fgb = not (np.any(fc_b) or np.any(gate_b))
    key = ("nc", skip_fgb)
    if key not in _CACHE:
        _CACHE[key] = _build(skip_fgb)
    nc = _CACHE[key]

    # column permutation: per 256-wide hidden group g the fp8 panel holds
    # [i|o|f|g] columns for hidden chunk g  (torch gate order i,f,g,o)
    wcat = np.concatenate([w_ih.T, w_hh.T], axis=0)     # (4096, 8192)
    bias = (b_ih + b_hh)
    perm8 = np.concatenate([np.concatenate([
        np.arange(0 * NHID + g * BS, 0 * NHID + (g + 1) * BS),
        np.arange(3 * NHID + g * BS, 3 * NHID + (g + 1) * BS),
        np.arange(1 * NHID + g * BS, 1 * NHID + (g + 1) * BS),
        np.arange(2 * NHID + g * BS, 2 * NHID + (g + 1) * BS)])
        for g in range(8)])
    w8_np = (wcat[:, perm8] * WSCALE).astype(E4)        # (4096, 8192)
    # -> [p, g, jj, plane, c]: k = 128*(2*JORDER[jj]+plane)+p, col = g*1024+c
    tmp = w8_np.reshape(32, 128, 8, 1024)               # [ktile, p, g, c]
    kts = [2 * JORDER[jj] + pl for jj in range(16) for pl in range(2)]
    w8d = np.ascontiguousarray(
        tmp[kts].reshape(16, 2, 128, 8, 1024).transpose(2, 3, 0, 1, 4))

    shared = {
        "wq": np.ascontiguousarray(
            wq_inp.reshape(NB, 2, 128, DKI).transpose(2, 1, 0, 3)),
        "wk1": np.ascontiguousarray(
            wk_inp[1].reshape(8, 128, DKI).transpose(1, 0, 2)),
        "wv1b": np.ascontiguousarray(
            wv_inp[1].reshape(8, 128, BS).transpose(1, 0, 2).astype(BF16)),
        "w8d": w8d,
        "bias8": (bias[perm8] * WSCALE).astype(BF16).reshape(1, 8192),
        "wqc": np.ascontiguousarray(
            wq_c.astype(BF16).reshape(NB, 2, 128, 128).transpose(2, 1, 0, 3)),
        "wkc": np.ascontiguousarray(
            wk_c.astype(BF16).reshape(NB, 2, 128, 128).transpose(2, 1, 0, 3)),
        "wvc": np.ascontiguousarray(
            wv_c.astype(BF16).reshape(NB, 2, 128, 128).transpose(2, 1, 0, 3)),
        "fgw": np.ascontiguousarray(
            np.concatenate([fc_w, gate_w], axis=1)).astype(BF16),
        "fgb": np.concatenate([fc_b, gate_b]).astype(BF16).reshape(1, 2 * BS),
    }

    in_maps = []
    for c in range(NCORES):
        rs = slice(c * BSH, (c + 1) * BSH)
        inpT_c = inp[rs].T.reshape(8, 128, BSH).transpose(1, 0, 2)
        hxT = hx[rs].T.reshape(16, 128, BSH).transpose(1, 0, 2)
        hxT8 = hxT.astype(E4)
        m = {
            "inpT": np.ascontiguousarray(inpT_c),
            "inpT_b": np.ascontiguousarray(inpT_c.astype(BF16)),
            "hxT_f": np.ascontiguousarray(hxT),
            "hxT_8": np.ascontiguousarray(hxT8),
            "hxE_8": np.ascontiguousarray(
                (hxT - hxT8.astype(np.float32)).astype(E4)),
            "hx_bm": np.ascontiguousarray(hx[rs]),
            "cx_bm": np.ascontiguousarray(cx[rs]),
        }
        m.update(shared)
        in_maps.append(m)

    from concourse.bass_utils import run_bass_kernel_spmd
    trace = bool(int(os.environ.get("BASS_KTRACE", "0")))
    res = run_bass_kernel_spmd(nc, in_maps, list(range(NCORES)), trace=trace)
    last_exec_time_ns = res.exec_time_ns
    last_results = res

    hx_full = np.empty((B, NHID), np.float32)
    cx_full = np.empty((B, NHID), np.float32)
    mask_full = np.empty((B, NHID), np.float32)
    for c in range(NCORES):
        rs = slice(c * BSH, (c + 1) * BSH)
        hx_full[rs] = res.results[c]["hx_out"]
        cx_full[rs] = res.results[c]["cx_out"]
        mask_full[rs] = np.repeat(res.results[c]["mask_out"], BS, axis=1)
    return hx_full, cx_full, mask_full


# revision 57
# speedup vs baseline: 1.2102x; 1.0392x over previous
"""Trainium2 Bass kernel for nn_BlocksCore (RIMs BlocksCore step).

Data-parallel over batch B=2048 across 8 NeuronCores (256 rows each),
parameters replicated. Per-core plan (v4):

  A. input attention: mask path (k1/q/s1/top-k) exact in f32; value path
     (v1T) in bf16; inp_flat^T produced feature-major as bf16 + fp8.
  B. LSTM gates all-fp8 with DoubleRow matmuls (K=256/instr), weights
     pre-scaled by 2^13 (fp8 subnormal avoidance), descaled in the PSUM
     activations; h-side fp8 residual pass on the [f|g] columns restores
     the c-path accuracy.  Processed per (hidden group g, batch half bt):
     one [128,4,256] PSUM tile per phase (3-deep rotation), weights
     fetched once per group as 16 contiguous [128,2048] lines.  Phase-C
     q/k/v projections for block g run inline right after group g's tail.
  C. communication attention: one 32-row score tile, single softmax,
     PE-expanded apply reading PSUM directly, gated residual + masked
     merge per block.

Outputs: hx_out/cx_out [256,2048] f32, mask_out [256,8] (host expands).
"""

import json
import os

import numpy as np
import ml_dtypes

BF16 = ml_dtypes.bfloat16
E4 = ml_dtypes.float8_e4m3

B = 2048
NCORES = 8
BSH = B // NCORES          # 256 batch rows per core
WSCALE = 2.0 ** 13         # fp8 weight pre-scale (keeps w out of subnormals)
WDESCALE = 2.0 ** -13
NINP = 1024
NHID = 2048
NB = 8                     # blocks
BS = 256                   # block size (NHID / NB)
DKI = 64                   # input-attention d_k

_CACHE = {}
last_exec_time_ns = None
last_results = None

# jj -> K-tile-pair order: hx pairs (8..15) first so phase B can start
# before phase A finishes producing inp_flat
JORDER = list(range(8, 16)) + list(range(8))

# ---------------------------------------------------------------------------
# BIR post-fix: this toolchain's core_v3 codegen supports only one sync-wait
# per CTRL-class instruction; hoist extras onto single-wait EventSemaphores.
# ---------------------------------------------------------------------------


def _fix_bir_json(bir_bytes: bytes) -> bytes:
    bir = json.loads(bir_bytes)
    for fn in bir.get("functions", []):
        for blk in fn.get("blocks", []):
            out = []
            for ins in blk.get("instructions", []):
                si = ins.get("sync_info") or {}
                waits = si.get("on_wait") or []
                if len(waits) > 1:
                    for j, w in enumerate(waits[:-1]):
                        out.append({
                            "name": f"{ins['name']}-w{j}",
                            "engine": ins["engine"],
                            "opcode": "EventSemaphore",
                            "ins": [],
                            "outs": [],
                            "sync_info": {"on_update": [], "on_wait": [w]},
                        })
                    si = dict(si)
                    si["on_wait"] = [waits[-1]]
                    ins = dict(ins)
                    ins["sync_info"] = si
                out.append(ins)
            blk["instructions"] = out
    return json.dumps(bir).encode()


def _install_bir_fix(nc):
    orig = nc.to_json_bytes

    def patched(*a, **k):
        return _fix_bir_json(orig(*a, **k))

    nc.to_json_bytes = patched


# ---------------------------------------------------------------------------
# Device kernel
# ---------------------------------------------------------------------------

def _build(skip_fgb):
    import concourse.bass as bass
    import concourse.tile as tile
    from concourse import mybir

    f32 = mybir.dt.float32
    bf16 = mybir.dt.bfloat16
    fp8 = mybir.dt.float8e4
    OP = mybir.AluOpType
    AF = mybir.ActivationFunctionType
    AX = mybir.AxisListType
    DR = mybir.MatmulPerfMode.DoubleRow

    nc = bass.Bass()

    # ---- I/O ------------------------------------------------------------
    inpT = nc.declare_dram_parameter("inpT", [128, 8, BSH], f32, isOutput=False)
    inpT_b = nc.declare_dram_parameter("inpT_b", [128, 8, BSH], bf16,
                                       isOutput=False)
    hxT_f = nc.declare_dram_parameter("hxT_f", [128, 16, BSH], f32,
                                      isOutput=False)
    hxT_8 = nc.declare_dram_parameter("hxT_8", [128, 16, BSH], fp8,
                                      isOutput=False)
    hxE_8 = nc.declare_dram_parameter("hxE_8", [128, 16, BSH], fp8,
                                      isOutput=False)
    hx_bm = nc.declare_dram_parameter("hx_bm", [BSH, NHID], f32, isOutput=False)
    cx_bm = nc.declare_dram_parameter("cx_bm", [BSH, NHID], f32, isOutput=False)
    wq = nc.declare_dram_parameter("wq", [128, 2, NB, DKI], f32, isOutput=False)
    wk1 = nc.declare_dram_parameter("wk1", [128, 8, DKI], f32, isOutput=False)
    wv1b = nc.declare_dram_parameter("wv1b", [128, 8, BS], bf16, isOutput=False)
    # LSTM weights, fp8*2^13: w8d[p, g, jj, plane, c] = W[k, g*1024+c] with
    # k = 128*(2*JORDER[jj]+plane)+p; per group g the 1024 columns are
    # [i|o|f|g] for hidden chunk g.
    w8d = nc.declare_dram_parameter("w8d", [128, 8, 16, 2, 1024], fp8,
                                    isOutput=False)
    bias8 = nc.declare_dram_parameter("bias8", [1, 8192], bf16, isOutput=False)
    wqc = nc.declare_dram_parameter("wqc", [128, 2, NB, 128], bf16,
                                    isOutput=False)
    wkc = nc.declare_dram_parameter("wkc", [128, 2, NB, 128], bf16,
                                    isOutput=False)
    wvc = nc.declare_dram_parameter("wvc", [128, 2, NB, 128], bf16,
                                    isOutput=False)
    fgw = nc.declare_dram_parameter("fgw", [128, 2 * BS], bf16, isOutput=False)
    fgb = nc.declare_dram_parameter("fgb", [1, 2 * BS], bf16, isOutput=False)
    hx_out = nc.declare_dram_parameter("hx_out", [BSH, NHID], f32, isOutput=True)
    cx_out = nc.declare_dram_parameter("cx_out", [BSH, NHID], f32, isOutput=True)
    mask_out = nc.declare_dram_parameter("mask_out", [BSH, NB], f32,
                                         isOutput=True)

    # ---- inline constants ----------------------------------------------
    hq_np = np.zeros((128, NB, 32), dtype=BF16)
    for d in range(128):
        for q in range(NB):
            hq_np[d, q, (d // 32) * 8 + q] = 1
    e32_np = np.zeros((32, NB, 128), dtype=BF16)
    for m in range(128):
        for q in range(NB):
            e32_np[(m // 32) * 8 + q, q, m] = 1
    # partition broadcaster: sel8[n', n, p] = (n' == n); a K=8 matmul with
    # lhsT=sel8[:, n, :] replicates row n of the rhs across 128 partitions
    sel8_np = np.zeros((8, NB, 128), dtype=BF16)
    for n in range(NB):
        sel8_np[n, n, :] = 1
    hqc = nc.inline_tensor(hq_np, "hqc")
    e32b = nc.inline_tensor(e32_np, "e32b")
    ones1c = nc.inline_tensor(np.ones((1, 128), dtype=BF16), "ones1c")
    sel8c = nc.inline_tensor(sel8_np, "sel8c")
    identc = nc.inline_tensor(np.eye(128, dtype=BF16), "identc")

    with tile.TileContext(nc) as tc:
        with tc.tile_pool(name="cp", bufs=1) as cp, \
             tc.tile_pool(name="pp", bufs=1) as pp:
            # ---- sync queue: A inputs needed earliest ------------------
            bias8_sb = cp.tile([1, 8192], bf16)
            nc.sync.dma_start(out=bias8_sb[:], in_=bias8[:])
            inpT_sb = pp.tile([128, 8, BSH], f32)
            nc.sync.dma_start(out=inpT_sb[:], in_=inpT[:])
            wk1_sb = pp.tile([128, 8, DKI], f32)
            nc.sync.dma_start(out=wk1_sb[:], in_=wk1[:])
            hxT8_sb = pp.tile([128, 16, BSH], fp8)
            nc.sync.dma_start(out=hxT8_sb[:], in_=hxT_8[:])
            hxE8_sb = pp.tile([128, 16, BSH], fp8)
            nc.sync.dma_start(out=hxE8_sb[:], in_=hxE_8[:])
            wv1_sb = pp.tile([128, 8, BS], bf16)
            nc.sync.dma_start(out=wv1_sb[:], in_=wv1b[:])
            inpTb_sb = pp.tile([128, 8, BSH], bf16)
            nc.sync.dma_start(out=inpTb_sb[:], in_=inpT_b[:])

            # ---- scalar queue: wq, then B weights join ------------------
            wq_sb = pp.tile([128, 2, NB, DKI], f32)
            nc.scalar.dma_start(out=wq_sb[:], in_=wq[:])

            # ---- gpsimd queue: hxTf first (mask path), then the rest ----
            hxTf_sb = pp.tile([128, 16, BSH], f32)
            nc.gpsimd.dma_start(out=hxTf_sb[:], in_=hxT_f[:])
            wqc_sb = cp.tile([128, 2, NB, 128], bf16)
            nc.gpsimd.dma_start(out=wqc_sb[:], in_=wqc[:])
            wkc_sb = cp.tile([128, 2, NB, 128], bf16)
            nc.gpsimd.dma_start(out=wkc_sb[:], in_=wkc[:])
            wvc_sb = cp.tile([128, 2, NB, 128], bf16)
            nc.gpsimd.dma_start(out=wvc_sb[:], in_=wvc[:])
            ident_sb = cp.tile([128, 128], bf16)
            nc.gpsimd.dma_start(out=ident_sb[:], in_=identc[:])
            sel8_sb = cp.tile([8, NB, 128], bf16)
            nc.gpsimd.dma_start(out=sel8_sb[:], in_=sel8c[:])
            ones1_sb = cp.tile([1, 128], bf16)
            nc.gpsimd.dma_start(out=ones1_sb[:], in_=ones1c[:])
            # cx/hx batch-major, loaded per-group-pair chunks in tail order
            cx_sb = [pp.tile([128, NHID], f32, tag=f"cx{bt}", name=f"cx{bt}")
                     for bt in range(2)]
            hx_sb = [pp.tile([128, NHID], f32, tag=f"hx{bt}", name=f"hx{bt}")
                     for bt in range(2)]
            for gp in range(1):
                sl = slice(gp * 512, (gp + 1) * 512)
                for bt in range(2):
                    nc.gpsimd.dma_start(out=cx_sb[bt][:, sl],
                                        in_=cx_bm[bt * 128:(bt + 1) * 128, sl])
                    nc.gpsimd.dma_start(out=hx_sb[bt][:, sl],
                                        in_=hx_bm[bt * 128:(bt + 1) * 128, sl])
            fgw_sb = cp.tile([128, 2 * BS], bf16)
            nc.gpsimd.dma_start(out=fgw_sb[:], in_=fgw[:])
            fgb_sb = cp.tile([1, 2 * BS], bf16)
            nc.gpsimd.dma_start(out=fgb_sb[:], in_=fgb[:])
            hq_sb = cp.tile([128, NB, 32], bf16)
            nc.gpsimd.dma_start(out=hq_sb[:], in_=hqc[:])
            e32_sb = cp.tile([32, NB, 128], bf16)
            nc.gpsimd.dma_start(out=e32_sb[:], in_=e32b[:])
            for gp in range(1, 4):
                sl = slice(gp * 512, (gp + 1) * 512)
                for bt in range(2):
                    nc.gpsimd.dma_start(out=cx_sb[bt][:, sl],
                                        in_=cx_bm[bt * 128:(bt + 1) * 128, sl])
                    nc.gpsimd.dma_start(out=hx_sb[bt][:, sl],
                                        in_=hx_bm[bt * 128:(bt + 1) * 128, sl])

            xt8_sb = pp.tile([128, 16, BSH], fp8)
            hnewT_sb = pp.tile([128, 16, BSH], bf16)
            mask_sb = [pp.tile([128, NB], f32, tag=f"mk{bt}", name=f"mk{bt}")
                      for bt in range(2)]
            sig_sb = [pp.tile([128, NB], bf16, tag=f"sg{bt}", name=f"sg{bt}")
                      for bt in range(2)]
            qc_sb = pp.tile([128, NB, BSH], bf16)
            kc_sb = pp.tile([128, NB, BSH], bf16)
            vc_sb = pp.tile([128, NB, BSH], bf16)

            # ---- phase A (mask path f32-exact) ---------------------------
            with tc.tile_pool(name="pa", bufs=1) as pa, \
                 tc.tile_pool(name="pa2", bufs=2) as pa2, \
                 tc.tile_pool(name="paps", bufs=1, space="PSUM") as paps:
                sigT_sb = pa.tile([8, BSH], bf16)
                for bt in range(2):
                    bsl = slice(bt * 128, (bt + 1) * 128)
                    k1_ps = paps.tile([128, DKI], f32, tag="k1")
                    for k in range(8):
                        nc.tensor.matmul(k1_ps[:], inpT_sb[:, k, bsl],
                                         wk1_sb[:, k, :],
                                         start=(k == 0), stop=(k == 7))
                    k1s = pa2.tile([128, DKI], f32, tag="k1s")
                    nc.vector.tensor_copy(k1s[:], k1_ps[:])

                    q_ps = paps.tile([128, NB, DKI], f32, tag="q")
                    for n in range(NB):
                        for s in range(2):
                            nc.tensor.matmul(q_ps[:, n, :],
                                             hxTf_sb[:, 2 * n + s, bsl],
                                             wq_sb[:, s, n, :],
                                             start=(s == 0), stop=(s == 1))
                    prod = pa2.tile([128, NB, DKI], f32, tag="prod")
                    k1a = k1s[:]
                    k1bc = bass.AP(tensor=k1a.tensor, offset=k1a.offset,
                                   ap=[k1a.ap[0], [0, NB], k1a.ap[1]])
                    nc.vector.tensor_tensor(prod[:], q_ps[:], k1bc, OP.mult)
                    s1 = pa2.tile([128, NB], f32, tag="s1")
                    nc.vector.reduce_sum(s1[:], prod[:], axis=AX.X)
                    nc.scalar.activation(sig_sb[bt][:], s1[:], AF.Sigmoid,
                                         scale=0.125)

                    # top-4 mask (rank counts fused via accum_out)
                    cnt = pa2.tile([128, NB], f32, tag="cnt")
                    tmp = pa2.tile([128, NB], f32, tag="tmp")
                    for n in range(NB):
                        nc.vector.tensor_scalar(tmp[:], s1[:], s1[:, n:n + 1],
                                                0.0, OP.is_gt, OP.add,
                                                accum_out=cnt[:, n:n + 1])
                    nc.vector.tensor_single_scalar(mask_sb[bt][:], cnt[:], 4.0,
                                                   OP.is_lt)
                    nc.gpsimd.dma_start(out=mask_out[bsl, :], in_=mask_sb[bt][:])
                    # sig^T half for the partition broadcast below
                    sgt = paps.tile([8, 128], bf16, tag="sgt")
                    nc.tensor.transpose(sgt[:], sig_sb[bt][:], ident_sb[:])
                    nc.vector.tensor_copy(sigT_sb[:, bsl], sgt[:])

                # v1^T = wv1^T @ inp^T in bf16 (value path; feeds fp8)
                v1T_sb = pa.tile([128, 2, BSH], bf16)
                for s in range(2):
                    v1T_ps = paps.tile([128, BSH], f32, tag="v1T")
                    for k in range(8):
                        nc.tensor.matmul(v1T_ps[:],
                                         wv1_sb[:, k, s * 128:(s + 1) * 128],
                                         inpTb_sb[:, k, :],
                                         start=(k == 0), stop=(k == 7))
                    nc.scalar.copy(v1T_sb[:, s, :], v1T_ps[:])

                # inp_flat^T = v1^T * broadcast(sig^T), fp8 straight out
                # of PSUM: 4 wide TTs instead of 16 mult+cast pairs
                with tc.tile_pool(name="pasg", bufs=2, space="PSUM") as pasg:
                    xa = xt8_sb[:]
                    st1 = xa.ap[1][0]
                    for nlo in (0, 4):
                        sgb = pasg.tile([128, 4, BSH], f32, tag="sgb")
                        for n in range(nlo, nlo + 4):
                            nc.tensor.matmul(sgb[:, n - nlo, :],
                                             sel8_sb[:, n, :],
                                             sigT_sb[:], start=True, stop=True)
                        for s in range(2):
                            sub = xt8_sb[:, 2 * nlo + s, :]
                            xt_v = bass.AP(tensor=sub.tensor,
                                           offset=sub.offset,
                                           ap=[sub.ap[0], [2 * st1, 4],
                                               sub.ap[-1]])
                            va = v1T_sb[:, s, :]
                            vbc = bass.AP(tensor=va.tensor, offset=va.offset,
                                          ap=[va.ap[0], [0, 4], va.ap[-1]])
                            nc.vector.tensor_tensor(xt_v, vbc, sgb[:],
                                                    OP.mult)

            # ---- phase B: LSTM groups, per (group, batch-half) ----------
            with tc.tile_pool(name="gps", bufs=1, space="PSUM") as gps, \
                 tc.tile_pool(name="prj", bufs=2, space="PSUM") as prj, \
                 tc.tile_pool(name="pw", bufs=9) as pw, \
                 tc.tile_pool(name="pb2", bufs=2) as pb2:
                w8t = {}
                for g in range(8):
                    for bt in range(2):
                        bsl = slice(bt * 128, (bt + 1) * 128)
                        gt = gps.tile([128, 4, BS], f32,
                                      tag=f"g{(2 * g + bt) % 3}",
                                      name=f"g{(2 * g + bt) % 3}")
                        nc.tensor.matmul(gt[:, 0:2, :], ones1_sb[:],
                                         bias8_sb[:, g * 1024:g * 1024 + 512],
                                         start=True, stop=False)
                        nc.tensor.matmul(gt[:, 2:4, :], ones1_sb[:],
                                         bias8_sb[:, g * 1024 + 512:
                                                  (g + 1) * 1024],
                                         start=True, stop=False)
                        for jj in range(16):
                            if bt == 0:
                                wt = pw.tile([128, 2, 1024], fp8, tag="w8t")
                                weng = nc.scalar if jj % 2 == 0 else nc.sync
                                weng.dma_start(out=wt[:], in_=w8d[:, g, jj, :, :])
                                w8t[jj] = wt
                            wt = w8t[jj]
                            st = (jj == 15)
                            if jj < 8:
                                t = 2 * jj
                                lhs8 = hxT8_sb[:, t:t + 2, bsl]
                                lhsE = hxE8_sb[:, t:t + 2, bsl]
                            else:
                                t = 2 * (jj - 8)
                                lhs8 = xt8_sb[:, t:t + 2, bsl]
                                lhsE = None
                            nc.tensor.matmul(gt[:, 0:2, :], lhs8,
                                             wt[:, :, 0:512],
                                             start=False, stop=st, perf_mode=DR)
                            nc.tensor.matmul(gt[:, 2:4, :], lhs8,
                                             wt[:, :, 512:1024],
                                             start=False, stop=st, perf_mode=DR)
                            if lhsE is not None:
                                nc.tensor.matmul(gt[:, 2:4, :], lhsE,
                                                 wt[:, :, 512:1024],
                                                 start=False, stop=False,
                                                 perf_mode=DR)
                        # ---- tail for (g, bt) ---------------------------
                        sl = slice(g * BS, (g + 1) * BS)
                        sio = pb2.tile([128, 2, BS], f32, tag="sio",
                                       name="sio", bufs=3)
                        nc.scalar.activation(sio[:], gt[:, 0:2, :],
                                             AF.Sigmoid, scale=WDESCALE)
                        sigf = pb2.tile([128, BS], f32, tag="sigf",
                                        name="sigf", bufs=3)
                        nc.scalar.activation(sigf[:], gt[:, 2, :],
                                             AF.Sigmoid, scale=WDESCALE)
                        tang = pb2.tile([128, BS], f32, tag="tang",
                                        name="tang", bufs=3)
                        nc.scalar.activation(tang[:], gt[:, 3, :],
                                             AF.Tanh, scale=WDESCALE)
                        t1 = pb2.tile([128, BS], f32, tag="t1", name="t1")
                        nc.vector.tensor_tensor(t1[:], sigf[:],
                                                cx_sb[bt][:, sl], OP.mult)
                        t2 = pb2.tile([128, BS], f32, tag="t2", name="t2")
                        nc.gpsimd.tensor_tensor(t2[:], sio[:, 0, :], tang[:],
                                                OP.mult)
                        cnew = pb2.tile([128, BS], f32, tag="cnew", name="cnew")
                        nc.vector.tensor_tensor(cnew[:], t1[:], t2[:], OP.add)
                        t3 = pb2.tile([128, BS], f32, tag="t3", name="t3")
                        nc.scalar.activation(t3[:], cnew[:], AF.Tanh)
                        hnb = pb2.tile([128, BS], bf16, tag="hnb", name="hnb")
                        nc.vector.tensor_tensor(hnb[:], sio[:, 1, :], t3[:],
                                                OP.mult)
                        hnw = pb2.tile([128, BS], f32, tag="hnw", name="hnw")
                        nc.gpsimd.tensor_tensor(hnw[:], sio[:, 1, :], t3[:],
                                                OP.mult)
                        dc = pb2.tile([128, BS], f32, tag="dc", name="dc")
                        nc.gpsimd.tensor_tensor(dc[:], cnew[:],
                                                cx_sb[bt][:, sl], OP.subtract)
                        co = pb2.tile([128, BS], f32, tag="co", name="co")
                        nc.vector.scalar_tensor_tensor(
                            co[:], dc[:], mask_sb[bt][:, g:g + 1],
                            cx_sb[bt][:, sl], OP.mult, OP.add)
                        nc.gpsimd.dma_start(
                            out=cx_out[bt * 128:(bt + 1) * 128, sl], in_=co[:])
                        for s in range(2):
                            teng = nc.sync if s == 0 else nc.scalar
                            teng.dma_start(
                                out=hnewT_sb[:, 2 * g + s,
                                             bt * 128:(bt + 1) * 128],
                                in_=hnb[:, s * 128:(s + 1) * 128],
                                transpose=True)
                        # d0 = h_new - hx, in place (merge shortcut)
                        nc.gpsimd.tensor_tensor(hnw[:], hnw[:],
                                                hx_sb[bt][:, sl], OP.subtract)
                        # base = mask*d0 + hx, in place in hx_sb
                        nc.vector.scalar_tensor_tensor(
                            hx_sb[bt][:, sl], hnw[:],
                            mask_sb[bt][:, g:g + 1], hx_sb[bt][:, sl],
                            OP.mult, OP.add)
                    # ---- inline phase-C projections for block g ---------
                    for wi, (wsb, osb) in enumerate(
                            ((wkc_sb, kc_sb), (wqc_sb, qc_sb),
                             (wvc_sb, vc_sb))):
                        ps = prj.tile([128, BSH], f32, tag="proj")
                        for s in range(2):
                            nc.tensor.matmul(ps[:], wsb[:, s, g, :],
                                             hnewT_sb[:, 2 * g + s, :],
                                             start=(s == 0), stop=(s == 1))
                        if wi % 2 == 0:
                            nc.scalar.copy(osb[:, g, :], ps[:])
                        else:
                            nc.vector.tensor_copy(osb[:, g, :], ps[:])

            # ============================ phase C ========================
            with tc.tile_pool(name="pc", bufs=1) as pc, \
                 tc.tile_pool(name="pctmp", bufs=2) as pctmp:
                at_sb = pc.tile([32, NB, BSH], bf16)
                with tc.tile_pool(name="psS", bufs=1, space="PSUM") as psS:
                    S = psS.tile([32, NB, BSH], f32, tag="S", name="S")
                    prg = {}
                    for q in (6, 7):
                        prg[q] = pctmp.tile([128, NB, BSH], bf16,
                                            tag=f"prg{q}", name=f"prg{q}",
                                            bufs=1)
                        qa = qc_sb[:, q, :]
                        qbc = bass.AP(tensor=qa.tensor, offset=qa.offset,
                                      ap=[qa.ap[0], [0, NB], qa.ap[-1]])
                        nc.gpsimd.tensor_tensor(prg[q][:], qbc, kc_sb[:],
                                                OP.mult)
                    for q in range(NB):
                        if q in prg:
                            pr = prg[q]
                            for kp in range(4):
                                nc.tensor.matmul(S[:, 2 * kp:2 * kp + 2, :],
                                                 hq_sb[:, q, :],
                                                 pr[:, 2 * kp:2 * kp + 2, :],
                                                 start=(q == 0), stop=(q == 7))
                        else:
                            pr = pctmp.tile([128, NB, BSH], bf16, tag="pr",
                                            name="pr", bufs=2)
                            qa = qc_sb[:, q, :]
                            qbc = bass.AP(tensor=qa.tensor, offset=qa.offset,
                                          ap=[qa.ap[0], [0, NB], qa.ap[-1]])
                            for half in range(2):
                                hs = slice(half * 4, half * 4 + 4)
                                qh = bass.AP(tensor=qa.tensor, offset=qa.offset,
                                             ap=[qa.ap[0], [0, 4], qa.ap[-1]])
                                nc.vector.tensor_tensor(pr[:, hs, :], qh,
                                                        kc_sb[:, hs, :],
                                                        OP.mult)
                                for kp in (2 * half, 2 * half + 1):
                                    nc.tensor.matmul(
                                        S[:, 2 * kp:2 * kp + 2, :],
                                        hq_sb[:, q, :],
                                        pr[:, 2 * kp:2 * kp + 2, :],
                                        start=(q == 0), stop=(q == 7))
                    ex = pc.tile([32, NB, BSH], bf16, tag="ex", name="ex")
                    nc.scalar.activation(ex[:], S[:], AF.Exp,
                                         scale=float(1.0 / np.sqrt(32.0)))
                    # denominator by bf16 tree adds (contiguous slices)
                    e1 = pctmp.tile([32, 4, BSH], bf16, tag="e1", name="e1")
                    nc.vector.tensor_tensor(e1[:], ex[:, 0:4, :], ex[:, 4:8, :],
                                            OP.add)
                    e2 = pctmp.tile([32, 2, BSH], bf16, tag="e2", name="e2")
                    nc.vector.tensor_tensor(e2[:], e1[:, 0:2, :], e1[:, 2:4, :],
                                            OP.add)
                    denom = pctmp.tile([32, BSH], f32, tag="denom",
                                       name="denom")
                    nc.vector.tensor_tensor(denom[:], e2[:, 0, :], e2[:, 1, :],
                                            OP.add)
                    recip = pctmp.tile([32, BSH], f32, tag="recip",
                                       name="recip")
                    nc.vector.reciprocal(recip[:], denom[:])
                    ra = recip[:]
                    rbc = bass.AP(tensor=ra.tensor, offset=ra.offset,
                                  ap=[ra.ap[0], [0, NB], ra.ap[-1]])
                    nc.vector.tensor_tensor(at_sb[:], ex[:], rbc, OP.mult)

                with tc.tile_pool(name="psU", bufs=1, space="PSUM") as psU, \
                     tc.tile_pool(name="psOG", bufs=2, space="PSUM") as psOG:
                    pend = []

                    def emit_merge(q, sgl, tanl):
                        qsl = slice(q * BS, (q + 1) * BS)
                        for bt in range(2):
                            mh = pctmp.tile([128, BS], f32, tag="mhq",
                                            name="mhq", bufs=4)
                            nc.vector.scalar_tensor_tensor(
                                mh[:], tanl[bt][:], mask_sb[bt][:, q:q + 1],
                                sgl[bt][:], OP.mult, OP.mult)
                            ho = pctmp.tile([128, BS], f32, tag="hoq",
                                            name="hoq", bufs=4)
                            nc.gpsimd.tensor_tensor(ho[:], mh[:],
                                                    hx_sb[bt][:, qsl], OP.add)
                            nc.gpsimd.dma_start(
                                out=hx_out[bt * 128:(bt + 1) * 128, qsl],
                                in_=ho[:])

                    for q in range(NB):
                        Ua = psU.tile([128, NB, BSH], f32, tag="Ua", name="Ua")
                        for kp in range(4):
                            nc.tensor.matmul(Ua[:, 2 * kp:2 * kp + 2, :],
                                             e32_sb[:, q, :],
                                             at_sb[:, 2 * kp:2 * kp + 2, :],
                                             start=True, stop=True)
                        m0 = pctmp.tile([128, NB, BSH], bf16, tag="m0",
                                        name="m0", bufs=2)
                        nc.vector.tensor_tensor(m0[:], Ua[:], vc_sb[:],
                                                OP.mult)
                        tr1 = pctmp.tile([128, 4, BSH], bf16, tag="tr1",
                                         name="tr1")
                        nc.vector.tensor_tensor(tr1[:], m0[:, 0:4, :],
                                                m0[:, 4:8, :], OP.add)
                        tr2 = pctmp.tile([128, 2, BSH], bf16, tag="tr2",
                                         name="tr2")
                        nc.vector.tensor_tensor(tr2[:], tr1[:, 0:2, :],
                                                tr1[:, 2:4, :], OP.add)
                        coutq = pctmp.tile([128, BSH], bf16, tag="coutq",
                                           name="coutq", bufs=2)
                        nc.vector.tensor_tensor(coutq[:], tr2[:, 0, :],
                                                tr2[:, 1, :], OP.add)
                        sgl, tanl = {}, {}
                        for bt in range(2):
                            csl = coutq[:, bt * 128:(bt + 1) * 128]
                            og = psOG.tile([128, 2 * BS], f32, tag="og",
                                           name="og")
                            nc.tensor.matmul(og[:], csl, fgw_sb[:],
                                             start=True, stop=skip_fgb)
                            if not skip_fgb:
                                nc.tensor.matmul(og[:], ones1_sb[:], fgb_sb[:],
                                                 start=False, stop=True)
                            tano = pctmp.tile([128, BS], f32, tag=f"tano{bt}",
                                              name=f"tano{bt}", bufs=2)
                            nc.scalar.activation(tano[:], og[:, 0:BS], AF.Tanh)
                            sg = pctmp.tile([128, BS], f32, tag=f"sgx{bt}",
                                            name=f"sgx{bt}", bufs=2)
                            nc.scalar.activation(sg[:], og[:, BS:2 * BS],
                                                 AF.Sigmoid)
                            sgl[bt], tanl[bt] = sg, tano
                        pend.append((q, sgl, tanl))
                        if len(pend) > 1:
                            emit_merge(*pend.pop(0))
                    while pend:
                        emit_merge(*pend.pop(0))

    _install_bir_fix(nc)
    return nc


# ---------------------------------------------------------------------------
# Host wrapper
# ---------------------------------------------------------------------------

def kernel(inp, hx, cx, wq_inp, wk_inp, wv_inp, w_ih, w_hh, b_ih, b_hh,
           wq_c, wk_c, wv_c, fc_w, fc_b, gate_w, gate_b, step=None):
    global last_exec_time_ns, last_results

    inp = np.asarray(inp, np.float32)
    hx = np.asarray(hx, np.float32)
    cx = np.asarray(cx, np.float32)
    wq_inp = np.asarray(wq_inp, np.float32)
    wk_inp = np.asarray(wk_inp, np.float32)
    wv_inp = np.asarray(wv_inp, np.float32)
    w_ih = np.asarray(w_ih, np.float32)
    w_hh = np.asarray(w_hh, np.float32)
    b_ih = np.asarray(b_ih, np.float32)
    b_hh = np.asarray(b_hh, np.float32)
    wq_c = np.asarray(wq_c, np.float32)
    wk_c = np.asarray(wk_c, np.float32)
    wv_c = np.asarray(wv_c, np.float32)
    fc_w = np.asarray(fc_w, np.float32)
    fc_b = np.asarray(fc_b, np.float32)
    gate_w = np.asarray(gate_w, np.float32)
    gate_b = np.asarray(gate_b, np.float32)

    skip_fgb = not (np.any(fc_b) or np.any(gate_b))
    key = ("nc", skip_fgb)
    if key not in _CACHE:
        _CACHE[key] = _build(skip_fgb)
    nc = _CACHE[key]

    # column permutation: per 256-wide hidden group g the fp8 panel holds
    # [i|o|f|g] columns for hidden chunk g  (torch gate order i,f,g,o)
    wcat = np.concatenate([w_ih.T, w_hh.T], axis=0)     # (4096, 8192)
    bias = (b_ih + b_hh)
    perm8 = np.concatenate([np.concatenate([
        np.arange(0 * NHID + g * BS, 0 * NHID + (g + 1) * BS),
        np.arange(3 * NHID + g * BS, 3 * NHID + (g + 1) * BS),
        np.arange(1 * NHID + g * BS, 1 * NHID + (g + 1) * BS),
        np.arange(2 * NHID + g * BS, 2 * NHID + (g + 1) * BS)])
        for g in range(8)])
    w8_np = (wcat[:, perm8] * WSCALE).astype(E4)        # (4096, 8192)
    # -> [p, g, jj, plane, c]: k = 128*(2*JORDER[jj]+plane)+p, col = g*1024+c
    tmp = w8_np.reshape(32, 128, 8, 1024)               # [ktile, p, g, c]
    kts = [2 * JORDER[jj] + pl for jj in range(16) for pl in range(2)]
    w8d = np.ascontiguousarray(
        tmp[kts].reshape(16, 2, 128, 8, 1024).transpose(2, 3, 0, 1, 4))

    shared = {
        "wq": np.ascontiguousarray(
            wq_inp.reshape(NB, 2, 128, DKI).transpose(2, 1, 0, 3)),
        "wk1": np.ascontiguousarray(
            wk_inp[1].reshape(8, 128, DKI).transpose(1, 0, 2)),
        "wv1b": np.ascontiguousarray(
            wv_inp[1].reshape(8, 128, BS).transpose(1, 0, 2).astype(BF16)),
        "w8d": w8d,
        "bias8": (bias[perm8] * WSCALE).astype(BF16).reshape(1, 8192),
        "wqc": np.ascontiguousarray(
            wq_c.astype(BF16).reshape(NB, 2, 128, 128).transpose(2, 1, 0, 3)),
        "wkc": np.ascontiguousarray(
            wk_c.astype(BF16).reshape(NB, 2, 128, 128).transpose(2, 1, 0, 3)),
        "wvc": np.ascontiguousarray(
            wv_c.astype(BF16).reshape(NB, 2, 128, 128).transpose(2, 1, 0, 3)),
        "fgw": np.ascontiguousarray(
            np.concatenate([fc_w, gate_w], axis=1)).astype(BF16),
        "fgb": np.concatenate([fc_b, gate_b]).astype(BF16).reshape(1, 2 * BS),
    }

    in_maps = []
    for c in range(NCORES):
        rs = slice(c * BSH, (c + 1) * BSH)
        inpT_c = inp[rs].T.reshape(8, 128, BSH).transpose(1, 0, 2)
        hxT = hx[rs].T.reshape(16, 128, BSH).transpose(1, 0, 2)
        hxT8 = hxT.astype(E4)
        m = {
            "inpT": np.ascontiguousarray(inpT_c),
            "inpT_b": np.ascontiguousarray(inpT_c.astype(BF16)),
            "hxT_f": np.ascontiguousarray(hxT),
            "hxT_8": np.ascontiguousarray(hxT8),
            "hxE_8": np.ascontiguousarray(
                (hxT - hxT8.astype(np.float32)).astype(E4)),
            "hx_bm": np.ascontiguousarray(hx[rs]),
            "cx_bm": np.ascontiguousarray(cx[rs]),
        }
        m.update(shared)
        in_maps.append(m)

    from concourse.bass_utils import run_bass_kernel_spmd
    trace = bool(int(os.environ.get("BASS_KTRACE", "0")))
    res = run_bass_kernel_spmd(nc, in_maps, list(range(NCORES)), trace=trace)
    last_exec_time_ns = res.exec_time_ns
    last_results = res

    hx_full = np.empty((B, NHID), np.float32)
    cx_full = np.empty((B, NHID), np.float32)
    mask_full = np.empty((B, NHID), np.float32)
    for c in range(NCORES):
        rs = slice(c * BSH, (c + 1) * BSH)
        hx_full[rs] = res.results[c]["hx_out"]
        cx_full[rs] = res.results[c]["cx_out"]
        mask_full[rs] = np.repeat(res.results[c]["mask_out"], BS, axis=1)
    return hx_full, cx_full, mask_full


# revision 58
# speedup vs baseline: 1.2518x; 1.0344x over previous
"""Trainium2 Bass kernel for nn_BlocksCore (RIMs BlocksCore step).

Data-parallel over batch B=2048 across 8 NeuronCores (256 rows each),
parameters replicated. Per-core plan (v4):

  A. input attention: mask path (k1/q/s1/top-k) exact in f32; value path
     (v1T) in bf16; inp_flat^T produced feature-major as bf16 + fp8.
  B. LSTM gates all-fp8 with DoubleRow matmuls (K=256/instr), weights
     pre-scaled by 2^13 (fp8 subnormal avoidance), descaled in the PSUM
     activations; h-side fp8 residual pass on the [f|g] columns restores
     the c-path accuracy.  Processed per (hidden group g, batch half bt):
     one [128,4,256] PSUM tile per phase (3-deep rotation), weights
     fetched once per group as 16 contiguous [128,2048] lines.  Phase-C
     q/k/v projections for block g run inline right after group g's tail.
  C. communication attention: one 32-row score tile, single softmax,
     PE-expanded apply reading PSUM directly, gated residual + masked
     merge per block.

Outputs: hx_out/cx_out [256,2048] f32, mask_out [256,8] (host expands).
"""

import json
import os

import numpy as np
import ml_dtypes

BF16 = ml_dtypes.bfloat16
E4 = ml_dtypes.float8_e4m3

B = 2048
NCORES = 8
BSH = B // NCORES          # 256 batch rows per core
WSCALE = 2.0 ** 13         # fp8 weight pre-scale (keeps w out of subnormals)
WDESCALE = 2.0 ** -13
NINP = 1024
NHID = 2048
NB = 8                     # blocks
BS = 256                   # block size (NHID / NB)
DKI = 64                   # input-attention d_k

_CACHE = {}
last_exec_time_ns = None
last_results = None

# jj -> K-tile-pair order: hx pairs (8..15) first so phase B can start
# before phase A finishes producing inp_flat
JORDER = list(range(8, 16)) + list(range(8))

# ---------------------------------------------------------------------------
# BIR post-fix: this toolchain's core_v3 codegen supports only one sync-wait
# per CTRL-class instruction; hoist extras onto single-wait EventSemaphores.
# ---------------------------------------------------------------------------


def _fix_bir_json(bir_bytes: bytes) -> bytes:
    bir = json.loads(bir_bytes)
    for fn in bir.get("functions", []):
        for blk in fn.get("blocks", []):
            out = []
            for ins in blk.get("instructions", []):
                si = ins.get("sync_info") or {}
                waits = si.get("on_wait") or []
                if len(waits) > 1:
                    for j, w in enumerate(waits[:-1]):
                        out.append({
                            "name": f"{ins['name']}-w{j}",
                            "engine": ins["engine"],
                            "opcode": "EventSemaphore",
                            "ins": [],
                            "outs": [],
                            "sync_info": {"on_update": [], "on_wait": [w]},
                        })
                    si = dict(si)
                    si["on_wait"] = [waits[-1]]
                    ins = dict(ins)
                    ins["sync_info"] = si
                out.append(ins)
            blk["instructions"] = out
    return json.dumps(bir).encode()


def _install_bir_fix(nc):
    orig = nc.to_json_bytes

    def patched(*a, **k):
        return _fix_bir_json(orig(*a, **k))

    nc.to_json_bytes = patched


# ---------------------------------------------------------------------------
# Device kernel
# ---------------------------------------------------------------------------

def _build(skip_fgb):
    import concourse.bass as bass
    import concourse.tile as tile
    from concourse import mybir

    f32 = mybir.dt.float32
    bf16 = mybir.dt.bfloat16
    fp8 = mybir.dt.float8e4
    OP = mybir.AluOpType
    AF = mybir.ActivationFunctionType
    AX = mybir.AxisListType
    DR = mybir.MatmulPerfMode.DoubleRow

    nc = bass.Bass()

    # ---- I/O ------------------------------------------------------------
    inpT = nc.declare_dram_parameter("inpT", [128, 8, BSH], f32, isOutput=False)
    inpT_b = nc.declare_dram_parameter("inpT_b", [128, 8, BSH], bf16,
                                       isOutput=False)
    hxT_f = nc.declare_dram_parameter("hxT_f", [128, 16, BSH], f32,
                                      isOutput=False)
    hxT_8 = nc.declare_dram_parameter("hxT_8", [128, 16, BSH], fp8,
                                      isOutput=False)
    hxE_8 = nc.declare_dram_parameter("hxE_8", [128, 16, BSH], fp8,
                                      isOutput=False)
    hx_bm = nc.declare_dram_parameter("hx_bm", [BSH, NHID], f32, isOutput=False)
    cx_bm = nc.declare_dram_parameter("cx_bm", [BSH, NHID], f32, isOutput=False)
    wq = nc.declare_dram_parameter("wq", [128, 2, NB, DKI], f32, isOutput=False)
    wk1 = nc.declare_dram_parameter("wk1", [128, 8, DKI], f32, isOutput=False)
    wv1b = nc.declare_dram_parameter("wv1b", [128, 8, BS], bf16, isOutput=False)
    # LSTM weights, fp8*2^13: w8d[p, g, jj, plane, c] = W[k, g*1024+c] with
    # k = 128*(2*JORDER[jj]+plane)+p; per group g the 1024 columns are
    # [i|o|f|g] for hidden chunk g.
    w8d = nc.declare_dram_parameter("w8d", [128, 8, 16, 2, 1024], fp8,
                                    isOutput=False)
    bias8 = nc.declare_dram_parameter("bias8", [1, 8192], bf16, isOutput=False)
    wqc = nc.declare_dram_parameter("wqc", [128, 2, NB, 128], bf16,
                                    isOutput=False)
    wkc = nc.declare_dram_parameter("wkc", [128, 2, NB, 128], bf16,
                                    isOutput=False)
    wvc = nc.declare_dram_parameter("wvc", [128, 2, NB, 128], bf16,
                                    isOutput=False)
    fgw = nc.declare_dram_parameter("fgw", [128, 2 * BS], bf16, isOutput=False)
    fgb = nc.declare_dram_parameter("fgb", [1, 2 * BS], bf16, isOutput=False)
    hx_out = nc.declare_dram_parameter("hx_out", [BSH, NHID], f32, isOutput=True)
    cx_out = nc.declare_dram_parameter("cx_out", [BSH, NHID], f32, isOutput=True)
    mask_out = nc.declare_dram_parameter("mask_out", [BSH, NB], f32,
                                         isOutput=True)

    # ---- inline constants ----------------------------------------------
    hq_np = np.zeros((128, NB, 32), dtype=BF16)
    for d in range(128):
        for q in range(NB):
            hq_np[d, q, (d // 32) * 8 + q] = 1
    e32_np = np.zeros((32, NB, 128), dtype=BF16)
    for m in range(128):
        for q in range(NB):
            e32_np[(m // 32) * 8 + q, q, m] = 1
    # partition broadcaster: sel8[n', n, p] = (n' == n); a K=8 matmul with
    # lhsT=sel8[:, n, :] replicates row n of the rhs across 128 partitions
    sel8_np = np.zeros((8, NB, 128), dtype=BF16)
    for n in range(NB):
        sel8_np[n, n, :] = 1
    hqc = nc.inline_tensor(hq_np, "hqc")
    e32b = nc.inline_tensor(e32_np, "e32b")
    ones1c = nc.inline_tensor(np.ones((1, 128), dtype=BF16), "ones1c")
    sel8c = nc.inline_tensor(sel8_np, "sel8c")
    identc = nc.inline_tensor(np.eye(128, dtype=BF16), "identc")

    with tile.TileContext(nc) as tc:
        with tc.tile_pool(name="cp", bufs=1) as cp, \
             tc.tile_pool(name="pp", bufs=1) as pp:
            # ---- sync queue: A inputs needed earliest ------------------
            bias8_sb = cp.tile([1, 8192], bf16)
            nc.sync.dma_start(out=bias8_sb[:], in_=bias8[:])
            inpT_sb = pp.tile([128, 8, BSH], f32)
            nc.sync.dma_start(out=inpT_sb[:], in_=inpT[:])
            wk1_sb = pp.tile([128, 8, DKI], f32)
            nc.sync.dma_start(out=wk1_sb[:], in_=wk1[:])
            hxT8_sb = pp.tile([128, 16, BSH], fp8)
            nc.sync.dma_start(out=hxT8_sb[:], in_=hxT_8[:])
            hxE8_sb = pp.tile([128, 16, BSH], fp8)
            nc.sync.dma_start(out=hxE8_sb[:], in_=hxE_8[:])
            wv1_sb = pp.tile([128, 8, BS], bf16)
            nc.sync.dma_start(out=wv1_sb[:], in_=wv1b[:])
            inpTb_sb = pp.tile([128, 8, BSH], bf16)
            nc.sync.dma_start(out=inpTb_sb[:], in_=inpT_b[:])

            # ---- scalar queue: wq, then B weights join ------------------
            wq_sb = pp.tile([128, 2, NB, DKI], f32)
            nc.scalar.dma_start(out=wq_sb[:], in_=wq[:])

            # ---- gpsimd queue: hxTf first (mask path), then the rest ----
            hxTf_sb = pp.tile([128, 16, BSH], f32)
            nc.gpsimd.dma_start(out=hxTf_sb[:], in_=hxT_f[:])
            wqc_sb = cp.tile([128, 2, NB, 128], bf16)
            nc.gpsimd.dma_start(out=wqc_sb[:], in_=wqc[:])
            wkc_sb = cp.tile([128, 2, NB, 128], bf16)
            nc.gpsimd.dma_start(out=wkc_sb[:], in_=wkc[:])
            wvc_sb = cp.tile([128, 2, NB, 128], bf16)
            nc.gpsimd.dma_start(out=wvc_sb[:], in_=wvc[:])
            ident_sb = cp.tile([128, 128], bf16)
            nc.gpsimd.dma_start(out=ident_sb[:], in_=identc[:])
            sel8_sb = cp.tile([8, NB, 128], bf16)
            nc.gpsimd.dma_start(out=sel8_sb[:], in_=sel8c[:])
            ones1_sb = cp.tile([1, 128], bf16)
            nc.gpsimd.dma_start(out=ones1_sb[:], in_=ones1c[:])
            # cx/hx batch-major, loaded per-group-pair chunks in tail order
            cx_sb = [pp.tile([128, NHID], f32, tag=f"cx{bt}", name=f"cx{bt}")
                     for bt in range(2)]
            hx_sb = [pp.tile([128, NHID], f32, tag=f"hx{bt}", name=f"hx{bt}")
                     for bt in range(2)]
            for gp in range(1):
                sl = slice(gp * 512, (gp + 1) * 512)
                for bt in range(2):
                    nc.gpsimd.dma_start(out=cx_sb[bt][:, sl],
                                        in_=cx_bm[bt * 128:(bt + 1) * 128, sl])
                    nc.gpsimd.dma_start(out=hx_sb[bt][:, sl],
                                        in_=hx_bm[bt * 128:(bt + 1) * 128, sl])
            fgw_sb = cp.tile([128, 2 * BS], bf16)
            nc.gpsimd.dma_start(out=fgw_sb[:], in_=fgw[:])
            fgb_sb = cp.tile([1, 2 * BS], bf16)
            nc.gpsimd.dma_start(out=fgb_sb[:], in_=fgb[:])
            hq_sb = cp.tile([128, NB, 32], bf16)
            nc.gpsimd.dma_start(out=hq_sb[:], in_=hqc[:])
            e32_sb = cp.tile([32, NB, 128], bf16)
            nc.gpsimd.dma_start(out=e32_sb[:], in_=e32b[:])
            for gp in range(1, 4):
                sl = slice(gp * 512, (gp + 1) * 512)
                for bt in range(2):
                    nc.gpsimd.dma_start(out=cx_sb[bt][:, sl],
                                        in_=cx_bm[bt * 128:(bt + 1) * 128, sl])
                    nc.gpsimd.dma_start(out=hx_sb[bt][:, sl],
                                        in_=hx_bm[bt * 128:(bt + 1) * 128, sl])

            xt8_sb = pp.tile([128, 16, BSH], fp8)
            hnewT_sb = pp.tile([128, 16, BSH], bf16)
            mask_sb = [pp.tile([128, NB], f32, tag=f"mk{bt}", name=f"mk{bt}")
                      for bt in range(2)]
            sig_sb = [pp.tile([128, NB], bf16, tag=f"sg{bt}", name=f"sg{bt}")
                      for bt in range(2)]
            qc_sb = pp.tile([128, NB, BSH], bf16)
            kc_sb = pp.tile([128, NB, BSH], bf16)
            vc_sb = pp.tile([128, NB, BSH], bf16)

            # ---- phase A (mask path f32-exact) ---------------------------
            with tc.tile_pool(name="pa", bufs=1) as pa, \
                 tc.tile_pool(name="pa2", bufs=2) as pa2, \
                 tc.tile_pool(name="paps", bufs=1, space="PSUM") as paps:
                sigT_sb = pa.tile([8, BSH], bf16)
                for bt in range(2):
                    bsl = slice(bt * 128, (bt + 1) * 128)
                    k1_ps = paps.tile([128, DKI], f32, tag="k1")
                    for k in range(8):
                        nc.tensor.matmul(k1_ps[:], inpT_sb[:, k, bsl],
                                         wk1_sb[:, k, :],
                                         start=(k == 0), stop=(k == 7))
                    k1s = pa2.tile([128, DKI], f32, tag="k1s")
                    nc.vector.tensor_copy(k1s[:], k1_ps[:])

                    q_ps = paps.tile([128, NB, DKI], f32, tag="q")
                    for n in range(NB):
                        for s in range(2):
                            nc.tensor.matmul(q_ps[:, n, :],
                                             hxTf_sb[:, 2 * n + s, bsl],
                                             wq_sb[:, s, n, :],
                                             start=(s == 0), stop=(s == 1))
                    prod = pa2.tile([128, NB, DKI], f32, tag="prod")
                    k1a = k1s[:]
                    k1bc = bass.AP(tensor=k1a.tensor, offset=k1a.offset,
                                   ap=[k1a.ap[0], [0, NB], k1a.ap[1]])
                    nc.vector.tensor_tensor(prod[:], q_ps[:], k1bc, OP.mult)
                    s1 = pa2.tile([128, NB], f32, tag="s1")
                    nc.vector.reduce_sum(s1[:], prod[:], axis=AX.X)
                    nc.scalar.activation(sig_sb[bt][:], s1[:], AF.Sigmoid,
                                         scale=0.125)

                    # top-4 mask (rank counts fused via accum_out)
                    cnt = pa2.tile([128, NB], f32, tag="cnt")
                    tmp = pa2.tile([128, NB], f32, tag="tmp")
                    for n in range(NB):
                        nc.vector.tensor_scalar(tmp[:], s1[:], s1[:, n:n + 1],
                                                0.0, OP.is_gt, OP.add,
                                                accum_out=cnt[:, n:n + 1])
                    nc.vector.tensor_single_scalar(mask_sb[bt][:], cnt[:], 4.0,
                                                   OP.is_lt)
                    nc.gpsimd.dma_start(out=mask_out[bsl, :], in_=mask_sb[bt][:])
                    # sig^T half for the partition broadcast below
                    sgt = paps.tile([8, 128], bf16, tag="sgt")
                    nc.tensor.transpose(sgt[:], sig_sb[bt][:], ident_sb[:])
                    nc.vector.tensor_copy(sigT_sb[:, bsl], sgt[:])

                # v1^T = wv1^T @ inp^T in bf16 (value path; feeds fp8)
                v1T_sb = pa.tile([128, 2, BSH], bf16)
                for s in range(2):
                    v1T_ps = paps.tile([128, BSH], f32, tag="v1T")
                    for k in range(8):
                        nc.tensor.matmul(v1T_ps[:],
                                         wv1_sb[:, k, s * 128:(s + 1) * 128],
                                         inpTb_sb[:, k, :],
                                         start=(k == 0), stop=(k == 7))
                    nc.scalar.copy(v1T_sb[:, s, :], v1T_ps[:])

                # inp_flat^T = v1^T * broadcast(sig^T), fp8 straight out
                # of PSUM: 4 wide TTs instead of 16 mult+cast pairs
                with tc.tile_pool(name="pasg", bufs=2, space="PSUM") as pasg:
                    xa = xt8_sb[:]
                    st1 = xa.ap[1][0]
                    for nlo in (0, 4):
                        sgb = pasg.tile([128, 4, BSH], f32, tag="sgb")
                        for n in range(nlo, nlo + 4):
                            nc.tensor.matmul(sgb[:, n - nlo, :],
                                             sel8_sb[:, n, :],
                                             sigT_sb[:], start=True, stop=True)
                        for s in range(2):
                            sub = xt8_sb[:, 2 * nlo + s, :]
                            xt_v = bass.AP(tensor=sub.tensor,
                                           offset=sub.offset,
                                           ap=[sub.ap[0], [2 * st1, 4],
                                               sub.ap[-1]])
                            va = v1T_sb[:, s, :]
                            vbc = bass.AP(tensor=va.tensor, offset=va.offset,
                                          ap=[va.ap[0], [0, 4], va.ap[-1]])
                            nc.vector.tensor_tensor(xt_v, vbc, sgb[:],
                                                    OP.mult)

            # ---- phase B: LSTM groups, per (group, batch-half) ----------
            with tc.tile_pool(name="gps", bufs=1, space="PSUM") as gps, \
                 tc.tile_pool(name="prj", bufs=2, space="PSUM") as prj, \
                 tc.tile_pool(name="pw", bufs=10) as pw, \
                 tc.tile_pool(name="pb2", bufs=2) as pb2:
                w8t = {}
                for g in range(8):
                    for bt in range(2):
                        bsl = slice(bt * 128, (bt + 1) * 128)
                        gt = gps.tile([128, 4, BS], f32,
                                      tag=f"g{(2 * g + bt) % 3}",
                                      name=f"g{(2 * g + bt) % 3}")
                        nc.tensor.matmul(gt[:, 0:2, :], ones1_sb[:],
                                         bias8_sb[:, g * 1024:g * 1024 + 512],
                                         start=True, stop=False)
                        nc.tensor.matmul(gt[:, 2:4, :], ones1_sb[:],
                                         bias8_sb[:, g * 1024 + 512:
                                                  (g + 1) * 1024],
                                         start=True, stop=False)
                        for jj in range(16):
                            if bt == 0:
                                wt = pw.tile([128, 2, 1024], fp8, tag="w8t")
                                weng = nc.scalar if jj % 2 == 0 else nc.sync
                                weng.dma_start(out=wt[:], in_=w8d[:, g, jj, :, :])
                                w8t[jj] = wt
                            wt = w8t[jj]
                            st = (jj == 15)
                            if jj < 8:
                                t = 2 * jj
                                lhs8 = hxT8_sb[:, t:t + 2, bsl]
                                lhsE = hxE8_sb[:, t:t + 2, bsl]
                            else:
                                t = 2 * (jj - 8)
                                lhs8 = xt8_sb[:, t:t + 2, bsl]
                                lhsE = None
                            nc.tensor.matmul(gt[:, 0:2, :], lhs8,
                                             wt[:, :, 0:512],
                                             start=False, stop=st, perf_mode=DR)
                            nc.tensor.matmul(gt[:, 2:4, :], lhs8,
                                             wt[:, :, 512:1024],
                                             start=False, stop=st, perf_mode=DR)
                            if lhsE is not None:
                                nc.tensor.matmul(gt[:, 2:4, :], lhsE,
                                                 wt[:, :, 512:1024],
                                                 start=False, stop=False,
                                                 perf_mode=DR)
                        # ---- tail for (g, bt) ---------------------------
                        sl = slice(g * BS, (g + 1) * BS)
                        sio = pb2.tile([128, 2, BS], f32, tag="sio",
                                       name="sio", bufs=3)
                        nc.scalar.activation(sio[:], gt[:, 0:2, :],
                                             AF.Sigmoid, scale=WDESCALE)
                        sigf = pb2.tile([128, BS], f32, tag="sigf",
                                        name="sigf", bufs=3)
                        nc.scalar.activation(sigf[:], gt[:, 2, :],
                                             AF.Sigmoid, scale=WDESCALE)
                        tang = pb2.tile([128, BS], f32, tag="tang",
                                        name="tang", bufs=3)
                        nc.scalar.activation(tang[:], gt[:, 3, :],
                                             AF.Tanh, scale=WDESCALE)
                        t1 = pb2.tile([128, BS], f32, tag="t1", name="t1")
                        nc.vector.tensor_tensor(t1[:], sigf[:],
                                                cx_sb[bt][:, sl], OP.mult)
                        t2 = pb2.tile([128, BS], f32, tag="t2", name="t2")
                        nc.gpsimd.tensor_tensor(t2[:], sio[:, 0, :], tang[:],
                                                OP.mult)
                        cnew = pb2.tile([128, BS], f32, tag="cnew", name="cnew")
                        nc.vector.tensor_tensor(cnew[:], t1[:], t2[:], OP.add)
                        t3 = pb2.tile([128, BS], f32, tag="t3", name="t3")
                        nc.scalar.activation(t3[:], cnew[:], AF.Tanh)
                        hnb = pb2.tile([128, BS], bf16, tag="hnb", name="hnb")
                        nc.vector.tensor_tensor(hnb[:], sio[:, 1, :], t3[:],
                                                OP.mult)
                        hnw = pb2.tile([128, BS], f32, tag="hnw", name="hnw")
                        nc.gpsimd.tensor_tensor(hnw[:], sio[:, 1, :], t3[:],
                                                OP.mult)
                        dc = pb2.tile([128, BS], f32, tag="dc", name="dc")
                        nc.gpsimd.tensor_tensor(dc[:], cnew[:],
                                                cx_sb[bt][:, sl], OP.subtract)
                        co = pb2.tile([128, BS], f32, tag="co", name="co")
                        nc.vector.scalar_tensor_tensor(
                            co[:], dc[:], mask_sb[bt][:, g:g + 1],
                            cx_sb[bt][:, sl], OP.mult, OP.add)
                        nc.gpsimd.dma_start(
                            out=cx_out[bt * 128:(bt + 1) * 128, sl], in_=co[:])
                        for s in range(2):
                            teng = nc.sync if s == 0 else nc.scalar
                            teng.dma_start(
                                out=hnewT_sb[:, 2 * g + s,
                                             bt * 128:(bt + 1) * 128],
                                in_=hnb[:, s * 128:(s + 1) * 128],
                                transpose=True)
                        # d0 = h_new - hx, in place (merge shortcut)
                        nc.gpsimd.tensor_tensor(hnw[:], hnw[:],
                                                hx_sb[bt][:, sl], OP.subtract)
                        # base = mask*d0 + hx, in place in hx_sb
                        nc.vector.scalar_tensor_tensor(
                            hx_sb[bt][:, sl], hnw[:],
                            mask_sb[bt][:, g:g + 1], hx_sb[bt][:, sl],
                            OP.mult, OP.add)
                    # ---- inline phase-C projections for block g ---------
                    for wi, (wsb, osb) in enumerate(
                            ((wkc_sb, kc_sb), (wqc_sb, qc_sb),
                             (wvc_sb, vc_sb))):
                        ps = prj.tile([128, BSH], f32, tag="proj")
                        for s in range(2):
                            nc.tensor.matmul(ps[:], wsb[:, s, g, :],
                                             hnewT_sb[:, 2 * g + s, :],
                                             start=(s == 0), stop=(s == 1))
                        if wi % 2 == 0:
                            nc.scalar.copy(osb[:, g, :], ps[:])
                        else:
                            nc.vector.tensor_copy(osb[:, g, :], ps[:])

            # ============================ phase C ========================
            with tc.tile_pool(name="pc", bufs=1) as pc, \
                 tc.tile_pool(name="pctmp", bufs=2) as pctmp:
                at_sb = pc.tile([32, NB, BSH], bf16)
                with tc.tile_pool(name="psS", bufs=1, space="PSUM") as psS:
                    S = psS.tile([32, NB, BSH], f32, tag="S", name="S")
                    prg = {}
                    for q in (6, 7):
                        prg[q] = pctmp.tile([128, NB, BSH], bf16,
                                            tag=f"prg{q}", name=f"prg{q}",
                                            bufs=1)
                        qa = qc_sb[:, q, :]
                        qbc = bass.AP(tensor=qa.tensor, offset=qa.offset,
                                      ap=[qa.ap[0], [0, NB], qa.ap[-1]])
                        nc.gpsimd.tensor_tensor(prg[q][:], qbc, kc_sb[:],
                                                OP.mult)
                    for q in range(NB):
                        if q in prg:
                            pr = prg[q]
                            for kp in range(4):
                                nc.tensor.matmul(S[:, 2 * kp:2 * kp + 2, :],
                                                 hq_sb[:, q, :],
                                                 pr[:, 2 * kp:2 * kp + 2, :],
                                                 start=(q == 0), stop=(q == 7))
                        else:
                            pr = pctmp.tile([128, NB, BSH], bf16, tag="pr",
                                            name="pr", bufs=2)
                            qa = qc_sb[:, q, :]
                            qbc = bass.AP(tensor=qa.tensor, offset=qa.offset,
                                          ap=[qa.ap[0], [0, NB], qa.ap[-1]])
                            for half in range(2):
                                hs = slice(half * 4, half * 4 + 4)
                                qh = bass.AP(tensor=qa.tensor, offset=qa.offset,
                                             ap=[qa.ap[0], [0, 4], qa.ap[-1]])
                                nc.vector.tensor_tensor(pr[:, hs, :], qh,
                                                        kc_sb[:, hs, :],
                                                        OP.mult)
                                for kp in (2 * half, 2 * half + 1):
                                    nc.tensor.matmul(
                                        S[:, 2 * kp:2 * kp + 2, :],
                                        hq_sb[:, q, :],
                                        pr[:, 2 * kp:2 * kp + 2, :],
                                        start=(q == 0), stop=(q == 7))
                    ex = pc.tile([32, NB, BSH], bf16, tag="ex", name="ex")
                    nc.scalar.activation(ex[:], S[:], AF.Exp,
                                         scale=float(1.0 / np.sqrt(32.0)))
                    # denominator by bf16 tree adds (contiguous slices)
                    e1 = pctmp.tile([32, 4, BSH], bf16, tag="e1", name="e1")
                    nc.vector.tensor_tensor(e1[:], ex[:, 0:4, :], ex[:, 4:8, :],
                                            OP.add)
                    e2 = pctmp.tile([32, 2, BSH], bf16, tag="e2", name="e2")
                    nc.vector.tensor_tensor(e2[:], e1[:, 0:2, :], e1[:, 2:4, :],
                                            OP.add)
                    denom = pctmp.tile([32, BSH], f32, tag="denom",
                                       name="denom")
                    nc.vector.tensor_tensor(denom[:], e2[:, 0, :], e2[:, 1, :],
                                            OP.add)
                    recip = pctmp.tile([32, BSH], f32, tag="recip",
                                       name="recip")
                    nc.vector.reciprocal(recip[:], denom[:])
                    ra = recip[:]
                    rbc = bass.AP(tensor=ra.tensor, offset=ra.offset,
                                  ap=[ra.ap[0], [0, NB], ra.ap[-1]])
                    nc.vector.tensor_tensor(at_sb[:], ex[:], rbc, OP.mult)

                with tc.tile_pool(name="psU", bufs=1, space="PSUM") as psU, \
                     tc.tile_pool(name="psOG", bufs=2, space="PSUM") as psOG:
                    pend = []

                    def emit_merge(q, sgl, tanl):
                        qsl = slice(q * BS, (q + 1) * BS)
                        for bt in range(2):
                            mh = pctmp.tile([128, BS], f32, tag="mhq",
                                            name="mhq", bufs=4)
                            nc.vector.scalar_tensor_tensor(
                                mh[:], tanl[bt][:], mask_sb[bt][:, q:q + 1],
                                sgl[bt][:], OP.mult, OP.mult)
                            ho = pctmp.tile([128, BS], f32, tag="hoq",
                                            name="hoq", bufs=4)
                            nc.gpsimd.tensor_tensor(ho[:], mh[:],
                                                    hx_sb[bt][:, qsl], OP.add)
                            nc.gpsimd.dma_start(
                                out=hx_out[bt * 128:(bt + 1) * 128, qsl],
                                in_=ho[:])

                    for q in range(NB):
                        Ua = psU.tile([128, NB, BSH], f32, tag="Ua", name="Ua")
                        for kp in range(4):
                            nc.tensor.matmul(Ua[:, 2 * kp:2 * kp + 2, :],
                                             e32_sb[:, q, :],
                                             at_sb[:, 2 * kp:2 * kp + 2, :],
                                             start=True, stop=True)
                        m0 = pctmp.tile([128, NB, BSH], bf16, tag="m0",
                                        name="m0", bufs=2)
                        nc.vector.tensor_tensor(m0[:], Ua[:], vc_sb[:],
                                                OP.mult)
                        tr1 = pctmp.tile([128, 4, BSH], bf16, tag="tr1",
                                         name="tr1")
                        nc.vector.tensor_tensor(tr1[:], m0[:, 0:4, :],
                                                m0[:, 4:8, :], OP.add)
                        tr2 = pctmp.tile([128, 2, BSH], bf16, tag="tr2",
                                         name="tr2")
                        nc.vector.tensor_tensor(tr2[:], tr1[:, 0:2, :],
                                                tr1[:, 2:4, :], OP.add)
                        coutq = pctmp.tile([128, BSH], bf16, tag="coutq",
                                           name="coutq", bufs=2)
                        nc.vector.tensor_tensor(coutq[:], tr2[:, 0, :],
                                                tr2[:, 1, :], OP.add)
                        sgl, tanl = {}, {}
                        for bt in range(2):
                            csl = coutq[:, bt * 128:(bt + 1) * 128]
                            og = psOG.tile([128, 2 * BS], f32, tag="og",
                                           name="og")
                            nc.tensor.matmul(og[:], csl, fgw_sb[:],
                                             start=True, stop=skip_fgb)
                            if not skip_fgb:
                                nc.tensor.matmul(og[:], ones1_sb[:], fgb_sb[:],
                                                 start=False, stop=True)
                            tano = pctmp.tile([128, BS], f32, tag=f"tano{bt}",
                                              name=f"tano{bt}", bufs=2)
                            nc.scalar.activation(tano[:], og[:, 0:BS], AF.Tanh)
                            sg = pctmp.tile([128, BS], f32, tag=f"sgx{bt}",
                                            name=f"sgx{bt}", bufs=2)
                            nc.scalar.activation(sg[:], og[:, BS:2 * BS],
                                                 AF.Sigmoid)
                            sgl[bt], tanl[bt] = sg, tano
                        pend.append((q, sgl, tanl))
                        if len(pend) > 1:
                            emit_merge(*pend.pop(0))
                    while pend:
                        emit_merge(*pend.pop(0))

    _install_bir_fix(nc)
    return nc


# ---------------------------------------------------------------------------
# Host wrapper
# ---------------------------------------------------------------------------

def kernel(inp, hx, cx, wq_inp, wk_inp, wv_inp, w_ih, w_hh, b_ih, b_hh,
           wq_c, wk_c, wv_c, fc_w, fc_b, gate_w, gate_b, step=None):
    global last_exec_time_ns, last_results

    inp = np.asarray(inp, np.float32)
    hx = np.asarray(hx, np.float32)
    cx = np.asarray(cx, np.float32)
    wq_inp = np.asarray(wq_inp, np.float32)
    wk_inp = np.asarray(wk_inp, np.float32)
    wv_inp = np.asarray(wv_inp, np.float32)
    w_ih = np.asarray(w_ih, np.float32)
    w_hh = np.asarray(w_hh, np.float32)
    b_ih = np.asarray(b_ih, np.float32)
    b_hh = np.asarray(b_hh, np.float32)
    wq_c = np.asarray(wq_c, np.float32)
    wk_c = np.asarray(wk_c, np.float32)
    wv_c = np.asarray(wv_c, np.float32)
    fc_w = np.asarray(fc_w, np.float32)
    fc_b = np.asarray(fc_b, np.float32)
    gate_w = np.asarray(gate_w, np.float32)
    gate_b = np.asarray(gate_b, np.float32)

    skip_fgb = not (np.any(fc_b) or np.any(gate_b))
    key = ("nc", skip_fgb)
    if key not in _CACHE:
        _CACHE[key] = _build(skip_fgb)
    nc = _CACHE[key]

    # column permutation: per 256-wide hidden group g the fp8 panel holds
    # [i|o|f|g] columns for hidden chunk g  (torch gate order i,f,g,o)
    wcat = np.concatenate([w_ih.T, w_hh.T], axis=0)     # (4096, 8192)
    bias = (b_ih + b_hh)
    perm8 = np.concatenate([np.concatenate([
        np.arange(0 * NHID + g * BS, 0 * NHID + (g + 1) * BS),
        np.arange(3 * NHID + g * BS, 3 * NHID + (g + 1) * BS),
        np.arange(1 * NHID + g * BS, 1 * NHID + (g + 1) * BS),
        np.arange(2 * NHID + g * BS, 2 * NHID + (g + 1) * BS)])
        for g in range(8)])
    w8_np = (wcat[:, perm8] * WSCALE).astype(E4)        # (4096, 8192)
    # -> [p, g, jj, plane, c]: k = 128*(2*JORDER[jj]+plane)+p, col = g*1024+c
    tmp = w8_np.reshape(32, 128, 8, 1024)               # [ktile, p, g, c]
    kts = [2 * JORDER[jj] + pl for jj in range(16) for pl in range(2)]
    w8d = np.ascontiguousarray(
        tmp[kts].reshape(16, 2, 128, 8, 1024).transpose(2, 3, 0, 1, 4))

    shared = {
        "wq": np.ascontiguousarray(
            wq_inp.reshape(NB, 2, 128, DKI).transpose(2, 1, 0, 3)),
        "wk1": np.ascontiguousarray(
            wk_inp[1].reshape(8, 128, DKI).transpose(1, 0, 2)),
        "wv1b": np.ascontiguousarray(
            wv_inp[1].reshape(8, 128, BS).transpose(1, 0, 2).astype(BF16)),
        "w8d": w8d,
        "bias8": (bias[perm8] * WSCALE).astype(BF16).reshape(1, 8192),
        "wqc": np.ascontiguousarray(
            wq_c.astype(BF16).reshape(NB, 2, 128, 128).transpose(2, 1, 0, 3)),
        "wkc": np.ascontiguousarray(
            wk_c.astype(BF16).reshape(NB, 2, 128, 128).transpose(2, 1, 0, 3)),
        "wvc": np.ascontiguousarray(
            wv_c.astype(BF16).reshape(NB, 2, 128, 128).transpose(2, 1, 0, 3)),
        "fgw": np.ascontiguousarray(
            np.concatenate([fc_w, gate_w], axis=1)).astype(BF16),
        "fgb": np.concatenate([fc_b, gate_b]).astype(BF16).reshape(1, 2 * BS),
    }

    in_maps = []
    for c in range(NCORES):
        rs = slice(c * BSH, (c + 1) * BSH)
        inpT_c = inp[rs].T.reshape(8, 128, BSH).transpose(1, 0, 2)
        hxT = hx[rs].T.reshape(16, 128, BSH).transpose(1, 0, 2)
        hxT8 = hxT.astype(E4)
        m = {
            "inpT": np.ascontiguousarray(inpT_c),
            "inpT_b": np.ascontiguousarray(inpT_c.astype(BF16)),
            "hxT_f": np.ascontiguousarray(hxT),
            "hxT_8": np.ascontiguousarray(hxT8),
            "hxE_8": np.ascontiguousarray(
                (hxT - hxT8.astype(np.float32)).astype(E4)),
            "hx_bm": np.ascontiguousarray(hx[rs]),
            "cx_bm": np.ascontiguousarray(cx[rs]),
        }
        m.update(shared)
        in_maps.append(m)

    from concourse.bass_utils import run_bass_kernel_spmd
    trace = bool(int(os.environ.get("BASS_KTRACE", "0")))
    res = run_bass_kernel_spmd(nc, in_maps, list(range(NCORES)), trace=trace)
    last_exec_time_ns = res.exec_time_ns
    last_results = res

    hx_full = np.empty((B, NHID), np.float32)
    cx_full = np.empty((B, NHID), np.float32)
    mask_full = np.empty((B, NHID), np.float32)
    for c in range(NCORES):
        rs = slice(c * BSH, (c + 1) * BSH)
        hx_full[rs] = res.results[c]["hx_out"]
        cx_full[rs] = res.results[c]["cx_out"]
        mask_full[rs] = np.repeat(res.results[c]["mask_out"], BS, axis=1)
    return hx_full, cx_full, mask_full
